# revision 9
# baseline (speedup 1.0000x reference)
"""Trainium2 Bass kernel for nn_CrossAttentionCondition (tensor-parallel v4).

v4 over v3:
- float16 activations/weights everywhere (4x finer mantissa than bf16, same
  PE rate); exp output + V in f16 with the exp shifted by -12 so e^s fits
  f16 range (softmax is shift-invariant; EX_F16=False falls back to bf16
  attention with no shift).
- Batched DMAs: one strided dma_start per tensor (~40 total vs ~310), each
  [128, chunks*cols]; SP issue time drops ~150us.
- KT/QT/attnT merged into head-major [128, 4*1024] tiles so each PE
  transpose needs ONE strided copy instead of four.
- Attention inner loop software-pipelined (scores run 2 tiles ahead of
  exp/PV); softmax denominator via a bf16/f16 pairwise tree of
  scalar_tensor_tensor adds (4x DVE mode) + one gpsimd partition reduce,
  instead of 8 full-rate f32 vector adds.
- o(qch0) matmuls interleaved into the qch1 attention phase; all o outputs
  staged f16 and written with one dma per token tile.

Sharding: 8 cores = 2 batches x 4 head-groups (4 heads / 512 dims each).
Column-sharded q/k/v projections, row-sharded o with host-side gather-add.
RMSNorm sum-of-squares over the full 2048 dims -> ONE AllReduce of 24
per-token-tile stat columns (8 k + 16 q), kicked right after the q/k
projections; v projections and ropes run behind it. RoPE pairs are
de-interleaved host-side; q/k are roped unnormalized (rope commutes with
the per-token scale), rk folds the 1/sqrt(hd) into the exp scale, rq is
applied on the roped q tiles after the collective lands.
"""

import numpy as np
import ml_dtypes

import concourse.bass as bass
import concourse.tile as tile
from concourse import bacc, mybir, bass_isa
from concourse.bass_utils import run_bass_kernel_spmd
from concourse.masks import make_identity

F16 = mybir.dt.float16
BF16 = mybir.dt.bfloat16
F32 = mybir.dt.float32
NPF16 = np.float16
NPBF16 = ml_dtypes.bfloat16

# exp/V dtype: f16 with shifted exp if True, bf16 unshifted fallback.
EX_F16 = True
EXDT = F16 if EX_F16 else BF16
NPEXDT = NPF16 if EX_F16 else NPBF16
EXP_SHIFT = -12.0 if EX_F16 else 0.0

DIM = 2048
H = 16
HD = 128
SC = 512
SR = 512
NKV = SC + SR
EPS = 1e-6
N_CORES = 8

KC = DIM // 128   # 16 contraction chunks
GH = 4            # heads per core
GD = GH * HD      # 512
NQT = 2048        # q tokens per core (full batch)
RG = [[0, 1, 2, 3], [4, 5, 6, 7]]
NST = 8 + 16      # stat columns: 8 k tiles + 16 q tiles
NKVT = NKV // 128  # 8
NQTT = NQT // 128  # 16

MUL = mybir.AluOpType.mult
ADD = mybir.AluOpType.add


def _body_tp(ctx, tc, dram):
    nc = tc.nc

    const = ctx.enter_context(tc.tile_pool(name="const", bufs=1))
    ident = const.tile([128, 128], F16, tag="ident")
    make_identity(nc, ident)
    eps_sb = const.tile([128, 1], F32, tag="eps")
    nc.vector.memset(eps_sb, EPS)
    eps_hd = const.tile([128, 1], F32, tag="epshd")
    nc.vector.memset(eps_hd, float(HD * EPS))
    bias_sh = const.tile([128, 1], F32, tag="bsh")
    nc.vector.memset(bias_sh, EXP_SHIFT)

    # persistent activation tiles (head-major layouts)
    ktp = ctx.enter_context(tc.tile_pool(name="ktp", bufs=1))
    qtp = ctx.enter_context(tc.tile_pool(name="qtp", bufs=2))
    vp = ctx.enter_context(tc.tile_pool(name="vp", bufs=NKVT))
    KT = ktp.tile([128, GH * NKV], F16, tag="kt", name="KT")
    QT = [qtp.tile([128, GH * 1024], F16, tag="qt", name=f"QT{c}")
          for c in range(2)]
    Vg = [vp.tile([128, GD], EXDT, tag="v", name=f"Vg{i}") for i in range(NKVT)]

    ss_pool = ctx.enter_context(tc.tile_pool(name="statss", bufs=1))
    ss_all = ss_pool.tile([128, NST], F32, tag="ss", name="ss_all")
    red = ss_pool.tile([128, NST], F32, tag="red", name="red")
    rk_all = ss_pool.tile([128, NKVT], F32, tag="rk", name="rk_all")
    rq_all = ss_pool.tile([128, NQTT], F32, tag="rq", name="rq_all")
    stat_pool = ctx.enter_context(tc.tile_pool(name="stat", bufs=2))

    wo_pool = ctx.enter_context(tc.tile_pool(name="wo", bufs=1))

    def load_big(pool, name, nchunk, ncol, tag, col0=None, colw=None):
        """One strided DMA: dram [nchunk*128, C] (or col slice) ->
        [128, nchunk*ncol]."""
        t = pool.tile([128, nchunk * ncol], F16, tag=tag)
        src = dram[name]
        if col0 is not None:
            src = src[:, col0:col0 + colw]
        nc.sync.dma_start(out=t.rearrange("p (c n) -> p c n", c=nchunk),
                          in_=src.rearrange("(c p) n -> p c n", p=128))
        return t

    def rms_stats(out_t, cols, bias, scale):
        std = stat_pool.tile([128, cols.shape[1]], F32, tag="std")
        nc.scalar.activation(
            out=std, in_=cols, func=mybir.ActivationFunctionType.Sqrt,
            bias=bias, scale=scale,
        )
        nc.vector.reciprocal(out=out_t, in_=std)

    def rope_tile(work_t, fr, fi):
        # in-place rope on the UNNORMALIZED tile; de-interleaved layout
        # (per head chunk [re(64) | im(64)]); all reads happen before writes.
        v4 = work_t.rearrange("p (h k i) -> p h k i", k=2, i=64)
        re, im = v4[:, :, 0, :], v4[:, :, 1, :]
        frv = fr.rearrange("p (h i) -> p h i", i=64)
        fiv = fi.rearrange("p (h i) -> p h i", i=64)
        t1 = rope_pool.tile([128, GH, 64], F16, tag="t1")
        t2 = rope_pool.tile([128, GH, 64], F16, tag="t2")
        t3 = rope_pool.tile([128, GH, 64], F16, tag="t3")
        t4 = rope_pool.tile([128, GH, 64], F16, tag="t4")
        nc.vector.tensor_mul(out=t1[:], in0=re, in1=frv)
        nc.vector.tensor_mul(out=t2[:], in0=im, in1=fiv)
        nc.vector.tensor_mul(out=t3[:], in0=re, in1=fiv)
        nc.vector.tensor_mul(out=t4[:], in0=im, in1=frv)
        nc.vector.tensor_sub(out=re, in0=t1[:], in1=t2[:])
        nc.vector.tensor_add(out=im, in0=t3[:], in1=t4[:])

    def transpose_tile(roped, dst3, ps_tr):
        """PE-transpose a [128 tok, GD] tile into 4 head blocks and store via
        ONE strided copy into dst3 ([128, GH, 128] view of a big tile)."""
        pt = ps_tr.tile([128, GD], F16, tag="tr")
        for d in range(GH):
            nc.tensor.transpose(
                pt[:, d * 128:(d + 1) * 128], roped[:, d * 128:(d + 1) * 128],
                ident[:],
            )
        nc.vector.tensor_copy(
            out=dst3, in_=pt.rearrange("p (d c) -> p d c", c=128))

    # ---------------- projections, one CC, ropes, q transposes ------------
    with (
        tc.tile_pool(name="ps_proj", bufs=3, space="PSUM") as ps_proj,
        tc.tile_pool(name="ps_tr", bufs=2, space="PSUM") as ps_tr,
        tc.tile_pool(name="srcp", bufs=1) as src_pool,
        tc.tile_pool(name="xp", bufs=2) as x_pool,
        tc.tile_pool(name="kw", bufs=NKVT) as kw_pool,
        tc.tile_pool(name="qw", bufs=NQTT) as qw_pool,
        tc.tile_pool(name="rope", bufs=2) as rope_pool_,
        tc.tile_pool(name="freq", bufs=1) as freq_pool,
        tc.tile_pool(name="wbig", bufs=2) as w_pool,
    ):
        rope_pool = rope_pool_
        kwork = [kw_pool.tile([128, GD], F16, tag="kw", name=f"kw{i}")
                 for i in range(NKVT)]
        qwork = [qw_pool.tile([128, GD], F16, tag="qw", name=f"qw{i}")
                 for i in range(NQTT)]
        def gproj(src, wt, posts):
            # src [128, KC*512] tokens-in-chunk, wt [128, KC*512]
            for i, post in enumerate(posts):
                ps = ps_proj.tile([128, GD], F32, tag="proj")
                for kc in range(KC):
                    nc.tensor.matmul(
                        ps[:],
                        src[:, kc * 512 + i * 128: kc * 512 + (i + 1) * 128],
                        wt[:, kc * 512:(kc + 1) * 512],
                        start=(kc == 0), stop=(kc == KC - 1),
                    )
                post(ps)

        def norm_post(work, col):
            def post(ps):
                nc.vector.tensor_copy(out=work[:], in_=ps[:])
                nc.scalar.activation(
                    out=ps[:], in_=ps[:],
                    func=mybir.ActivationFunctionType.Square,
                    accum_out=ss_all[:, col:col + 1],
                )
            return post

        def v_post(tt):
            def post(ps):
                nc.scalar.activation(
                    out=Vg[tt][:], in_=ps[:],
                    func=mybir.ActivationFunctionType.Copy,
                )
            return post

        def k_rope_transpose(tt):
            if tt < 4:
                fr, fi = frc, fic
                c0 = tt * 256
            else:
                fr, fi = frr, fir
                c0 = (tt - 4) * 256
            rope_tile(kwork[tt], fr[:, c0:c0 + 256], fi[:, c0:c0 + 256])
            dst = KT.rearrange("p (d kv) -> p d kv", d=GH)[
                :, :, tt * 128:(tt + 1) * 128]
            transpose_tile(kwork[tt], dst, ps_tr)

        # k projections (cam then render), stats into ss_all[:, 0..7]
        cam_src = load_big(src_pool, "camT", KC, 512, "cam")
        wk = load_big(w_pool, "wkTg", KC, 512, "w")
        ren_src = load_big(src_pool, "renT", KC, 512, "ren")
        wkr = load_big(w_pool, "wkrTg", KC, 512, "w")
        frc = load_big(freq_pool, "frc", 4, 256, "frc")
        fic = load_big(freq_pool, "fic", 4, 256, "fic")
        frr = load_big(freq_pool, "frr", 4, 256, "frr")
        fir = load_big(freq_pool, "fir", 4, 256, "fir")
        gproj(cam_src, wk, [norm_post(kwork[t], t) for t in range(4)])
        gproj(ren_src, wkr, [norm_post(kwork[4 + t], 4 + t) for t in range(4)])

        # q projections, stats into ss_all[:, 8..23]; k ropes+transposes
        # interleave behind them; v/wo weight streams prefetch late.
        wq = load_big(w_pool, "wqTg", KC, 512, "w")
        frq = load_big(freq_pool, "frq", NQTT, 256, "frq")
        fiq = load_big(freq_pool, "fiq", NQTT, 256, "fiq")
        wv = wvr = None
        for ch in range(4):
            xsrc = load_big(x_pool, "xT", KC, 512, "x",
                            col0=ch * 512, colw=512)
            gproj(xsrc, wq,
                  [norm_post(qwork[ch * 4 + i], 8 + ch * 4 + i)
                   for i in range(4)])
            if ch == 0:
                for tt in range(4):
                    k_rope_transpose(tt)
            elif ch == 1:
                for tt in range(4, NKVT):
                    k_rope_transpose(tt)
            elif ch == 2:
                wv = load_big(w_pool, "wvTg", KC, 512, "w")

        # ONE collective for all 24 stat columns
        nc.sync.dma_start(
            out=dram["cc_in"].rearrange("(p j) -> p j", p=128), in_=ss_all[:]
        )
        nc.gpsimd.collective_compute(
            "AllReduce", mybir.AluOpType.add,
            ins=[dram["cc_in"]], outs=[dram["cc_out"]],
            replica_groups=RG,
        )
        nc.sync.dma_start(
            out=red[:], in_=dram["cc_out"].rearrange("(p j) -> p j", p=128)
        )

        # v projections and q ropes stream behind the collective
        gproj(cam_src, wv, [v_post(t) for t in range(4)])
        wvr = load_big(w_pool, "wvrTg", KC, 512, "w")
        for j in range(8):
            rope_tile(qwork[j], frq[:, j * 256:(j + 1) * 256],
                      fiq[:, j * 256:(j + 1) * 256])
        gproj(ren_src, wvr, [v_post(4 + t) for t in range(4)])
        for j in range(8, NQTT):
            rope_tile(qwork[j], frq[:, j * 256:(j + 1) * 256],
                      fiq[:, j * 256:(j + 1) * 256])
        wo_big = wo_pool.tile([128, GH * DIM], F16, tag="wob", name="wo_big")
        nc.sync.dma_start(
            out=wo_big.rearrange("p (c n) -> p c n", c=GH),
            in_=dram["woTg"].rearrange("(c p) n -> p c n", p=128))

        # post-collective: batched stats, then q normalize + transpose
        # (emitted in j order so attention can start on the first tiles).
        # rk folds the 1/sqrt(hd) score scale:
        #   SCORE_SCALE / sqrt(ss/DIM + EPS) = 1 / sqrt(ss*HD/DIM + HD*EPS)
        rms_stats(rk_all, red[:, 0:NKVT], eps_hd[:], float(HD) / DIM)
        rms_stats(rq_all, red[:, NKVT:NST], eps_sb[:], 1.0 / DIM)

        for j in range(NQTT):
            nc.vector.tensor_scalar_mul(out=qwork[j][:], in0=qwork[j][:],
                                        scalar1=rq_all[:, j:j + 1])
            qch, jj = divmod(j, 8)
            dst = QT[qch].rearrange("p (d c) -> p d c", d=GH)[
                :, :, jj * 128:(jj + 1) * 128]
            transpose_tile(qwork[j], dst, ps_tr)

    # ---------------- attention + o ----------------
    atp = ctx.enter_context(tc.tile_pool(name="atp", bufs=2))
    AT = [atp.tile([128, GH * 1024], F16, tag="at", name=f"AT{c}")
          for c in range(2)]
    expp = ctx.enter_context(tc.tile_pool(name="expp", bufs=5))
    l1p = ctx.enter_context(tc.tile_pool(name="l1p", bufs=5))
    l2p = ctx.enter_context(tc.tile_pool(name="l2p", bufs=3))
    accp = ctx.enter_context(tc.tile_pool(name="accp", bufs=2))
    stage_pool = ctx.enter_context(tc.tile_pool(name="stage", bufs=3))

    def attn_head(qch, h, ps_sc, ps_at, fill=None):
        """Software-pipelined flash-style head: scores run 2 kv-tiles ahead
        of exp/PV; denominator = pairwise tree of 4x-mode adds + one gpsimd
        partition reduce."""
        at_ps = [ps_at.tile([128, 512], F32, tag="at", name=f"at{qch}_{h}_{i}")
                 for i in range(2)]
        sc = [None] * NKVT
        ex = [None] * NKVT
        l1 = [None] * 4

        def emit_sc(kvt):
            s = ps_sc.tile([128, 1024], F32, tag="sc")
            for hf in range(2):
                nc.tensor.matmul(
                    s[:, hf * 512:(hf + 1) * 512],
                    KT[:, h * NKV + kvt * 128: h * NKV + (kvt + 1) * 128],
                    QT[qch][:, h * 1024 + hf * 512: h * 1024 + (hf + 1) * 512],
                    start=True, stop=True,
                )
            sc[kvt] = s

        emit_sc(0)
        if fill is not None:
            fill()
        emit_sc(1)
        for kvt in range(NKVT):
            e = expp.tile([128, 1024], EXDT, tag="exp")
            nc.scalar.activation(
                out=e[:], in_=sc[kvt][:],
                func=mybir.ActivationFunctionType.Exp,
                scale=rk_all[:, kvt:kvt + 1], bias=bias_sh[:],
            )
            ex[kvt] = e
            if kvt + 2 < NKVT:
                emit_sc(kvt + 2)
            for hf in range(2):
                nc.tensor.matmul(
                    at_ps[hf][:], Vg[kvt][:, h * 128:(h + 1) * 128],
                    e[:, hf * 512:(hf + 1) * 512],
                    start=(kvt == 0), stop=(kvt == NKVT - 1),
                )
            if kvt % 2 == 1:
                t = l1p.tile([128, 1024], EXDT, tag="l1")
                nc.vector.scalar_tensor_tensor(
                    out=t[:], in0=ex[kvt - 1][:], scalar=1.0, in1=ex[kvt][:],
                    op0=MUL, op1=ADD)
                l1[kvt // 2] = t
        l2a = l2p.tile([128, 1024], EXDT, tag="l2")
        l2b = l2p.tile([128, 1024], EXDT, tag="l2")
        nc.vector.scalar_tensor_tensor(out=l2a[:], in0=l1[0][:], scalar=1.0,
                                       in1=l1[1][:], op0=MUL, op1=ADD)
        nc.vector.scalar_tensor_tensor(out=l2b[:], in0=l1[2][:], scalar=1.0,
                                       in1=l1[3][:], op0=MUL, op1=ADD)
        acc = accp.tile([128, 1024], F32, tag="acc")
        nc.vector.scalar_tensor_tensor(out=acc[:], in0=l2a[:], scalar=1.0,
                                       in1=l2b[:], op0=MUL, op1=ADD)
        den = accp.tile([128, 1024], F32, tag="den")
        nc.gpsimd.partition_all_reduce(den[:], acc[:], channels=128,
                                       reduce_op=bass_isa.ReduceOp.add)
        nc.vector.reciprocal(out=den[:], in_=den[:])
        for hf in range(2):
            nc.vector.tensor_mul(
                out=AT[qch][:, h * 1024 + hf * 512: h * 1024 + (hf + 1) * 512],
                in0=at_ps[hf][:], in1=den[:, hf * 512:(hf + 1) * 512],
            )

    def o_tile(qch, tj, ot, ps_o, stage, copy_eng):
        """One [128,512] slice of the o projection for token tile tj of qch."""
        ps = ps_o.tile([128, 512], F32, tag="o")
        for hc in range(GH):
            nc.tensor.matmul(
                ps[:],
                AT[qch][:, hc * 1024 + tj * 128: hc * 1024 + (tj + 1) * 128],
                wo_big[:, hc * DIM + ot * 512: hc * DIM + (ot + 1) * 512],
                start=(hc == 0), stop=(hc == GH - 1),
            )
        if copy_eng == "act":
            nc.scalar.activation(out=stage[:, ot * 512:(ot + 1) * 512],
                                 in_=ps[:],
                                 func=mybir.ActivationFunctionType.Copy)
        else:
            nc.vector.tensor_copy(out=stage[:, ot * 512:(ot + 1) * 512],
                                  in_=ps[:])

    def o_token(qch, tj, ps_o, copy_eng):
        tt = qch * 8 + tj
        stage = stage_pool.tile([128, DIM], F16, tag="stage")
        for ot in range(4):
            o_tile(qch, tj, ot, ps_o, stage, copy_eng)
        nc.sync.dma_start(
            out=dram["out"][tt * 128:(tt + 1) * 128, :], in_=stage[:])

    with (
        tc.tile_pool(name="ps_scA", bufs=2, space="PSUM") as ps_sc,
        tc.tile_pool(name="ps_atA", bufs=4, space="PSUM") as ps_at,
    ):
        for h in range(GH):
            attn_head(0, h, ps_sc, ps_at)

    with (
        tc.tile_pool(name="ps_scC", bufs=2, space="PSUM") as ps_sc,
        tc.tile_pool(name="ps_atC", bufs=2, space="PSUM") as ps_at,
        tc.tile_pool(name="ps_oC", bufs=2, space="PSUM") as ps_oC,
    ):
        def fill_for(h):
            if h == 0:
                return None

            def fill():
                for tj in (2 * (h - 1), 2 * (h - 1) + 1):
                    o_token(0, tj, ps_oC, "dve")
            return fill

        for h in range(GH):
            attn_head(1, h, ps_sc, ps_at, fill=fill_for(h))
        for tj in (6, 7):
            o_token(0, tj, ps_oC, "dve")

    with tc.tile_pool(name="ps_oD", bufs=6, space="PSUM") as ps_oD:
        for tj in range(8):
            o_token(1, tj, ps_oD, "act" if tj % 2 else "dve")


_NC_CACHE = {}


def build_program():
    import os
    key = (os.environ.get("KERNEL_TIMING_REPS", "0"),)
    if key in _NC_CACHE:
        return _NC_CACHE[key]
    from contextlib import ExitStack

    nc = bacc.Bacc(
        "TRN2", target_bir_lowering=False, debug=False,
        enable_asserts=True, num_devices=N_CORES,
    )
    dram = {}
    specs = [
        ("xT", [DIM, NQT], F16),
        ("camT", [DIM, SC], F16),
        ("renT", [DIM, SR], F16),
        ("wqTg", [DIM, GD], F16),
        ("wkTg", [DIM, GD], F16),
        ("wvTg", [DIM, GD], F16),
        ("wkrTg", [DIM, GD], F16),
        ("wvrTg", [DIM, GD], F16),
        ("woTg", [GD, DIM], F16),
        ("frq", [NQT, GH * 64], F16),
        ("fiq", [NQT, GH * 64], F16),
        ("frc", [SC, GH * 64], F16),
        ("fic", [SC, GH * 64], F16),
        ("frr", [SR, GH * 64], F16),
        ("fir", [SR, GH * 64], F16),
    ]
    for name, shape, dt in specs:
        dram[name] = nc.dram_tensor(name, shape, dt, kind="ExternalInput").ap()
    dram["cc_in"] = nc.dram_tensor("cc_in", [NST * 128], F32, kind="Internal").ap()
    dram["cc_out"] = nc.dram_tensor("cc_out", [NST * 128], F32, kind="Internal").ap()
    dram["out"] = nc.dram_tensor("out", [NQT, DIM], F16, kind="ExternalOutput").ap()

    timing_reps = int(os.environ.get("KERNEL_TIMING_REPS", "0"))
    with tile.TileContext(nc) as tc:
        for _ in range(max(1, timing_reps)):
            with ExitStack() as ctx:
                _body_tp(ctx, tc, dram)
    nc.compile()
    _NC_CACHE[key] = nc
    return nc


def _expand_freqs(freqs, nh=GH):
    # freqs [s, 64, 2] -> fr, fi each [s, nh*64] (per-head repeat)
    fr = np.ascontiguousarray(
        np.broadcast_to(freqs[:, None, :, 0], (freqs.shape[0], nh, 64))
    ).reshape(freqs.shape[0], nh * 64)
    fi = np.ascontiguousarray(
        np.broadcast_to(freqs[:, None, :, 1], (freqs.shape[0], nh, 64))
    ).reshape(freqs.shape[0], nh * 64)
    return (np.ascontiguousarray(fr.astype(NPF16)),
            np.ascontiguousarray(fi.astype(NPF16)))


def _rope_perm():
    # de-interleave (re, im) pairs within each head's 128 dims:
    # new col h*128 + s*64 + i  <-  old col h*128 + 2*i + s
    perm = np.empty(GD, np.int64)
    for h in range(GH):
        for i in range(64):
            for s in range(2):
                perm[h * 128 + s * 64 + i] = h * 128 + 2 * i + s
    return perm


def make_in_maps_tp(x, cam_emb, render_emb, freqs_x, freqs_cam, freqs_render,
                    wq, bq, wk, bk, wv, bv, wkr, bkr, wvr, bvr, wo, bo, gq, gk):
    for b in (bq, bk, bv, bkr, bvr, bo):
        assert np.abs(np.asarray(b)).max() == 0.0, "nonzero bias unsupported"
    assert np.allclose(np.asarray(gq), 1.0) and np.allclose(np.asarray(gk), 1.0), \
        "non-unit rmsnorm gains unsupported"

    def wT(w):
        return np.asarray(w).T.astype(NPF16)

    wqT, wkT, wvT = wT(wq), wT(wk), wT(wv)
    wkrT, wvrT, woT = wT(wkr), wT(wvr), wT(wo)
    frq, fiq = _expand_freqs(np.asarray(freqs_x))
    frc, fic = _expand_freqs(np.asarray(freqs_cam))
    frr, fir = _expand_freqs(np.asarray(freqs_render))
    perm = _rope_perm()

    x = np.asarray(x)
    cam = np.asarray(cam_emb)
    ren = np.asarray(render_emb)
    xT = [np.ascontiguousarray(x[b].T.astype(NPF16)) for b in range(2)]
    camT = [np.ascontiguousarray(cam[b].T.astype(NPF16)) for b in range(2)]
    renT = [np.ascontiguousarray(ren[b].T.astype(NPF16)) for b in range(2)]
    in_maps = []
    for c in range(N_CORES):
        b, g = divmod(c, 4)
        gs = slice(g * GD, (g + 1) * GD)
        m = {
            "xT": xT[b], "camT": camT[b], "renT": renT[b],
            "wqTg": np.ascontiguousarray(wqT[:, gs][:, perm]),
            "wkTg": np.ascontiguousarray(wkT[:, gs][:, perm]),
            "wvTg": np.ascontiguousarray(wvT[:, gs]),
            "wkrTg": np.ascontiguousarray(wkrT[:, gs][:, perm]),
            "wvrTg": np.ascontiguousarray(wvrT[:, gs]),
            "woTg": np.ascontiguousarray(woT[gs, :]),
            "frq": frq, "fiq": fiq,
            "frc": frc, "fic": fic, "frr": frr, "fir": fir,
        }
        in_maps.append(m)
    return in_maps


def kernel(**inputs):
    nc = build_program()
    in_maps = make_in_maps_tp(**inputs)
    res = run_bass_kernel_spmd(nc, in_maps, core_ids=list(range(N_CORES)))
    x = np.asarray(inputs["x"])
    out = np.empty((x.shape[0], x.shape[1], DIM), dtype=np.float32)
    for b in range(2):
        acc = res.results[4 * b]["out"].astype(np.float32)
        for g in range(1, 4):
            acc = acc + res.results[4 * b + g]["out"].astype(np.float32)
        out[b] = acc
    out += np.asarray(inputs["bo"])[None, None, :]
    return out


def _make_timed_runner(nc, in_maps):
    """Reusable jitted SPMD callable with device-resident inputs."""
    import jax
    from jax.experimental.shard_map import shard_map
    from jax.sharding import Mesh, PartitionSpec, NamedSharding
    from concourse import bass2jax, mybir as mb

    bass2jax.install_neuronx_cc_hook()

    in_names, out_names, out_avals = [], [], []
    partition_name = nc.partition_id_tensor.name if nc.partition_id_tensor else None
    for alloc in nc.m.functions[0].allocations:
        if not isinstance(alloc, mb.MemoryLocationSet):
            continue
        name = alloc.memorylocations[0].name
        if alloc.kind == "ExternalInput":
            if name != partition_name:
                in_names.append(name)
        elif alloc.kind == "ExternalOutput":
            shape = tuple(alloc.tensor_shape)
            dtype = mb.dt.np(alloc.dtype)
            out_names.append(name)
            out_avals.append(jax.core.ShapedArray(shape, dtype))
    n_params = len(in_names)
    all_names = list(in_names) + list(out_names)
    if partition_name is not None:
        all_names.append(partition_name)

    def _body(*args):
        operands = list(args)
        if partition_name is not None:
            operands.append(bass2jax.partition_id_tensor())
        outs = bass2jax._bass_exec_p.bind(
            *operands,
            out_avals=tuple(out_avals),
            in_names=tuple(all_names),
            out_names=tuple(out_names),
            lowering_input_output_aliases=(),
            sim_require_finite=True,
            sim_require_nnan=True,
            nc=nc,
        )
        return tuple(outs)

    devices = jax.devices()[:N_CORES]
    mesh = Mesh(np.asarray(devices), ("core",))
    in_specs = (PartitionSpec("core"),) * (n_params + len(out_names))
    out_specs = (PartitionSpec("core"),) * len(out_names)
    sharded = jax.jit(
        shard_map(_body, mesh=mesh, in_specs=in_specs, out_specs=out_specs,
                  check_rep=False),
        keep_unused=True,
    )
    sharding = NamedSharding(mesh, PartitionSpec("core"))
    concat_in = [
        jax.device_put(
            np.concatenate([np.asarray(in_maps[c][nm]) for c in range(N_CORES)],
                           axis=0),
            sharding,
        )
        for nm in in_names
    ]
    for av in out_avals:
        concat_in.append(
            jax.device_put(
                np.zeros((N_CORES * av.shape[0], *av.shape[1:]), av.dtype), sharding
            )
        )
    return sharded, concat_in


def bench(inputs, iters=10):
    """Return per-execution device time in ns, amortized over `iters` runs."""
    import time
    import jax

    nc = build_program()
    in_maps = make_in_maps_tp(**inputs)
    fn, dev_in = _make_timed_runner(nc, in_maps)
    outs = fn(*dev_in)
    jax.block_until_ready(outs)
    t0 = time.perf_counter()
    for _ in range(iters):
        outs = fn(*dev_in)
    jax.block_until_ready(outs)
    dt = (time.perf_counter() - t0) / iters
    return dt * 1e9


# revision 29
# speedup vs baseline: 1.0769x; 1.0769x over previous
"""Trainium2 Bass kernel for nn_CrossAttentionCondition (tensor-parallel v4).

v4 over v3:
- float16 activations/weights everywhere (4x finer mantissa than bf16, same
  PE rate); exp output + V in f16 with the exp shifted by -12 so e^s fits
  f16 range (softmax is shift-invariant; EX_F16=False falls back to bf16
  attention with no shift).
- Batched DMAs: one strided dma_start per tensor (~40 total vs ~310), each
  [128, chunks*cols]; SP issue time drops ~150us.
- KT/QT/attnT merged into head-major [128, 4*1024] tiles so each PE
  transpose needs ONE strided copy instead of four.
- Attention inner loop software-pipelined (scores run 2 tiles ahead of
  exp/PV); softmax denominator via a bf16/f16 pairwise tree of
  scalar_tensor_tensor adds (4x DVE mode) + one gpsimd partition reduce,
  instead of 8 full-rate f32 vector adds.
- o(qch0) matmuls interleaved into the qch1 attention phase; all o outputs
  staged f16 and written with one dma per token tile.

Sharding: 8 cores = 2 batches x 4 head-groups (4 heads / 512 dims each).
Column-sharded q/k/v projections, row-sharded o with host-side gather-add.
RMSNorm sum-of-squares over the full 2048 dims -> ONE AllReduce of 24
per-token-tile stat columns (8 k + 16 q), kicked right after the q/k
projections; v projections and ropes run behind it. RoPE pairs are
de-interleaved host-side; q/k are roped unnormalized (rope commutes with
the per-token scale), rk folds the 1/sqrt(hd) into the exp scale, rq is
applied on the roped q tiles after the collective lands.
"""

import numpy as np
import ml_dtypes

import concourse.bass as bass
import concourse.tile as tile
from concourse import bacc, mybir, bass_isa
from concourse.bass_utils import run_bass_kernel_spmd
from concourse.masks import make_identity

F16 = mybir.dt.float16
BF16 = mybir.dt.bfloat16
F32 = mybir.dt.float32
NPF16 = np.float16
NPBF16 = ml_dtypes.bfloat16

# exp/V dtype: f16 with shifted exp if True, bf16 unshifted fallback.
EX_F16 = True
EXDT = F16 if EX_F16 else BF16
NPEXDT = NPF16 if EX_F16 else NPBF16
EXP_SHIFT = -12.0 if EX_F16 else 0.0

DIM = 2048
H = 16
HD = 128
SC = 512
SR = 512
NKV = SC + SR
EPS = 1e-6
N_CORES = 8

KC = DIM // 128   # 16 contraction chunks
GH = 4            # heads per core
GD = GH * HD      # 512
NQT = 2048        # q tokens per core (full batch)
RG = [[0, 1, 2, 3], [4, 5, 6, 7]]
NST = 8 + 16      # stat columns: 8 k tiles + 16 q tiles
NKVT = NKV // 128  # 8
NQTT = NQT // 128  # 16

MUL = mybir.AluOpType.mult
ADD = mybir.AluOpType.add


def _body_tp(ctx, tc, dram):
    nc = tc.nc

    const = ctx.enter_context(tc.tile_pool(name="const", bufs=1))
    ident = const.tile([128, 128], F16, tag="ident")
    make_identity(nc, ident)
    eps_sb = const.tile([128, 1], F32, tag="eps")
    nc.vector.memset(eps_sb, EPS)
    eps_hd = const.tile([128, 1], F32, tag="epshd")
    nc.vector.memset(eps_hd, float(HD * EPS))
    bias_sh = const.tile([128, 1], F32, tag="bsh")
    nc.vector.memset(bias_sh, EXP_SHIFT)
    # dummy sqrt pins the initial act table to sqrt_and_others (which also
    # contains Square/Copy), so the post-collective stats don't pay a table
    # load on the critical path.
    warm = const.tile([128, 1], F32, tag="warm")
    nc.scalar.activation(out=warm, in_=eps_sb[:],
                         func=mybir.ActivationFunctionType.Sqrt)

    # persistent activation tiles (head-major layouts)
    ktp = ctx.enter_context(tc.tile_pool(name="ktp", bufs=1))
    qtp = ctx.enter_context(tc.tile_pool(name="qtp", bufs=2))
    vp = ctx.enter_context(tc.tile_pool(name="vp", bufs=NKVT))
    KT = ktp.tile([128, GH * NKV], F16, tag="kt", name="KT")
    QT = [qtp.tile([128, GH * 1024], F16, tag="qt", name=f"QT{c}")
          for c in range(2)]
    Vg = [vp.tile([128, GD], EXDT, tag="v", name=f"Vg{i}") for i in range(NKVT)]

    ss_pool = ctx.enter_context(tc.tile_pool(name="statss", bufs=1))
    ss_all = ss_pool.tile([128, NST], F32, tag="ss", name="ss_all")
    red = ss_pool.tile([128, NST], F32, tag="red", name="red")
    rk_all = ss_pool.tile([128, NKVT], F32, tag="rk", name="rk_all")
    rq_all = ss_pool.tile([128, NQTT], F32, tag="rq", name="rq_all")
    stat_pool = ctx.enter_context(tc.tile_pool(name="stat", bufs=2))

    wo_pool = ctx.enter_context(tc.tile_pool(name="wo", bufs=1))

    def load_big(pool, name, nchunk, ncol, tag, col0=None, colw=None,
                 chunk0=0):
        """One strided DMA: dram rows [chunk0*128, (chunk0+nchunk)*128) (and
        optional col slice) -> [128, nchunk*ncol]."""
        t = pool.tile([128, nchunk * ncol], F16, tag=tag)
        src = dram[name][chunk0 * 128:(chunk0 + nchunk) * 128, :]
        if col0 is not None:
            src = src[:, col0:col0 + colw]
        nc.sync.dma_start(out=t.rearrange("p (c n) -> p c n", c=nchunk),
                          in_=src.rearrange("(c p) n -> p c n", p=128))
        return t

    def rms_stats(out_t, cols, bias, scale):
        std = stat_pool.tile([128, cols.shape[1]], F32, tag="std")
        nc.scalar.activation(
            out=std, in_=cols, func=mybir.ActivationFunctionType.Sqrt,
            bias=bias, scale=scale,
        )
        nc.vector.reciprocal(out=out_t, in_=std)

    def rope_tile(work_t, fr, fi):
        # in-place rope on the UNNORMALIZED tile; de-interleaved layout
        # (per head chunk [re(64) | im(64)]); all reads happen before writes.
        v4 = work_t.rearrange("p (h k i) -> p h k i", k=2, i=64)
        re, im = v4[:, :, 0, :], v4[:, :, 1, :]
        frv = fr.rearrange("p (h i) -> p h i", i=64)
        fiv = fi.rearrange("p (h i) -> p h i", i=64)
        t1 = rope_pool.tile([128, GH, 64], F16, tag="t1")
        t2 = rope_pool.tile([128, GH, 64], F16, tag="t2")
        t3 = rope_pool.tile([128, GH, 64], F16, tag="t3")
        t4 = rope_pool.tile([128, GH, 64], F16, tag="t4")
        nc.vector.tensor_mul(out=t1[:], in0=re, in1=frv)
        nc.vector.tensor_mul(out=t2[:], in0=im, in1=fiv)
        nc.vector.tensor_mul(out=t3[:], in0=re, in1=fiv)
        nc.vector.tensor_mul(out=t4[:], in0=im, in1=frv)
        nc.vector.tensor_sub(out=re, in0=t1[:], in1=t2[:])
        nc.vector.tensor_add(out=im, in0=t3[:], in1=t4[:])

    def transpose_tile(roped, dst3, ps_tr, eng="dve"):
        """PE-transpose a [128 tok, GD] tile into 4 head blocks and store via
        ONE strided copy into dst3 ([128, GH, 128] view of a big tile)."""
        pt = ps_tr.tile([128, GD], F16, tag="tr")
        for d in range(GH):
            nc.tensor.transpose(
                pt[:, d * 128:(d + 1) * 128], roped[:, d * 128:(d + 1) * 128],
                ident[:],
            )
        src3 = pt.rearrange("p (d c) -> p d c", c=128)
        if eng == "act":
            nc.scalar.activation(out=dst3, in_=src3,
                                 func=mybir.ActivationFunctionType.Copy)
        else:
            nc.vector.tensor_copy(out=dst3, in_=src3)

    # ---------------- projections, one CC, ropes, q transposes ------------
    with (
        tc.tile_pool(name="ps_proj", bufs=5, space="PSUM") as ps_proj,
        tc.tile_pool(name="ps_tr", bufs=2, space="PSUM") as ps_tr,
        tc.tile_pool(name="srcp", bufs=1) as src_pool,
        tc.tile_pool(name="xp", bufs=3) as x_pool,
        tc.tile_pool(name="kw", bufs=NKVT) as kw_pool,
        tc.tile_pool(name="qw", bufs=NQTT) as qw_pool,
        tc.tile_pool(name="rope", bufs=1) as rope_pool_,
        tc.tile_pool(name="freq", bufs=1) as freq_pool,
        tc.tile_pool(name="wbig", bufs=2) as w_pool,
    ):
        rope_pool = rope_pool_
        kwork = [kw_pool.tile([128, GD], F16, tag="kw", name=f"kw{i}")
                 for i in range(NKVT)]
        qwork = [qw_pool.tile([128, GD], F16, tag="qw", name=f"qw{i}")
                 for i in range(NQTT)]
        def big_ap(t):
            # accessor over one [128, KC*512] tile
            def src(kc, i):
                return t[:, kc * 512 + i * 128: kc * 512 + (i + 1) * 128]

            def wt(kc):
                return t[:, kc * 512:(kc + 1) * 512]
            return src, wt

        def half_ap(t0, t1):
            # accessor over two [128, 8*512] half tiles
            def src(kc, i):
                t = t0 if kc < 8 else t1
                c = kc % 8
                return t[:, c * 512 + i * 128: c * 512 + (i + 1) * 128]

            def wt(kc):
                t = t0 if kc < 8 else t1
                c = kc % 8
                return t[:, c * 512:(c + 1) * 512]
            return src, wt

        def gproj(src, wt, posts):
            for i, post in enumerate(posts):
                ps = ps_proj.tile([128, GD], F32, tag="proj")
                for kc in range(KC):
                    nc.tensor.matmul(
                        ps[:], src(kc, i), wt(kc),
                        start=(kc == 0), stop=(kc == KC - 1),
                    )
                post(ps)

        def gproj_first(src, wt, posts):
            # kc-split variant: runs chunks 0..7 for every output tile before
            # touching chunks 8..15, so compute starts after only the first
            # half of the src/weight DMAs has landed.
            ps_list = [ps_proj.tile([128, GD], F32, tag="proj",
                                    name=f"psf{i}")
                       for i in range(len(posts))]
            for half in (0, 1):
                for i in range(len(posts)):
                    for kc in range(half * 8, half * 8 + 8):
                        nc.tensor.matmul(
                            ps_list[i], src(kc, i), wt(kc),
                            start=(kc == 0), stop=(kc == KC - 1),
                        )
            for i, post in enumerate(posts):
                post(ps_list[i])

        def norm_post(work, col):
            def post(ps):
                nc.vector.tensor_copy(out=work[:], in_=ps[:])
                nc.scalar.activation(
                    out=ps[:], in_=ps[:],
                    func=mybir.ActivationFunctionType.Square,
                    accum_out=ss_all[:, col:col + 1],
                )
            return post

        def v_post(tt):
            def post(ps):
                nc.scalar.activation(
                    out=Vg[tt][:], in_=ps[:],
                    func=mybir.ActivationFunctionType.Copy,
                )
            return post

        def k_rope_transpose(tt):
            if tt < 4:
                fr, fi = frc, fic
                c0 = tt * 256
            else:
                fr, fi = frr, fir
                c0 = (tt - 4) * 256
            rope_tile(kwork[tt], fr[:, c0:c0 + 256], fi[:, c0:c0 + 256])
            dst = KT.rearrange("p (d kv) -> p d kv", d=GH)[
                :, :, tt * 128:(tt + 1) * 128]
            transpose_tile(kwork[tt], dst, ps_tr)

        # k projections (cam then render), stats into ss_all[:, 0..7].
        # cam/wk are loaded in halves and the first projection is kc-split
        # so PE starts after only the first half of the DMAs has landed.
        cam0 = load_big(src_pool, "camT", 8, 512, "cam0")
        wk0 = load_big(w_pool, "wkTg", 8, 512, "wh")
        cam1 = load_big(src_pool, "camT", 8, 512, "cam1", chunk0=8)
        wk1 = load_big(w_pool, "wkTg", 8, 512, "wh", chunk0=8)
        ren_src = load_big(src_pool, "renT", KC, 512, "ren")
        wkr = load_big(w_pool, "wkrTg", KC, 512, "w")
        cam_ap, _ = half_ap(cam0, cam1)
        _, wk_ap = half_ap(wk0, wk1)
        ren_ap, wkr_ap = big_ap(ren_src), big_ap(wkr)[1]
        gproj_first(cam_ap, wk_ap,
                    [norm_post(kwork[t], t) for t in range(4)])
        def load_x(ch):
            h0 = load_big(x_pool, "xT", 8, 512, "x",
                          col0=ch * 512, colw=512)
            h1 = load_big(x_pool, "xT", 8, 512, "x",
                          col0=ch * 512, colw=512, chunk0=8)
            return half_ap(h0, h1)[0]

        wq = load_big(w_pool, "wqTg", KC, 512, "w")
        x0 = load_x(0)
        gproj(ren_ap[0], wkr_ap,
              [norm_post(kwork[4 + t], 4 + t) for t in range(4)])

        # q projections, stats into ss_all[:, 8..23]; k ropes+transposes
        # interleave behind them; v/wo weight streams prefetch late.
        frc = load_big(freq_pool, "frc", 4, 256, "frc")
        fic = load_big(freq_pool, "fic", 4, 256, "fic")
        frr = load_big(freq_pool, "frr", 4, 256, "frr")
        fir = load_big(freq_pool, "fir", 4, 256, "fir")
        frq = load_big(freq_pool, "frq", NQTT, 256, "frq")
        fiq = load_big(freq_pool, "fiq", NQTT, 256, "fiq")
        wq_ap = big_ap(wq)
        wv = wvr = None
        for ch in range(4):
            xs_ap = x0 if ch == 0 else load_x(ch)
            gproj(xs_ap, wq_ap[1],
                  [norm_post(qwork[ch * 4 + i], 8 + ch * 4 + i)
                   for i in range(4)])
            if ch == 0:
                for tt in range(4):
                    k_rope_transpose(tt)
            elif ch == 1:
                for tt in range(4, NKVT):
                    k_rope_transpose(tt)
            elif ch == 2:
                wv = load_big(w_pool, "wvTg", KC, 512, "w")

        # ONE collective for all 24 stat columns
        nc.sync.dma_start(
            out=dram["cc_in"].rearrange("(p j) -> p j", p=128), in_=ss_all[:]
        )
        nc.gpsimd.collective_compute(
            "AllReduce", mybir.AluOpType.add,
            ins=[dram["cc_in"]], outs=[dram["cc_out"]],
            replica_groups=RG,
        )
        nc.sync.dma_start(
            out=red[:], in_=dram["cc_out"].rearrange("(p j) -> p j", p=128)
        )

        # v projections and q ropes stream behind the collective
        gproj(cam_ap, big_ap(wv)[1], [v_post(t) for t in range(4)])
        wvr = load_big(w_pool, "wvrTg", KC, 512, "w")
        for j in range(8):
            rope_tile(qwork[j], frq[:, j * 256:(j + 1) * 256],
                      fiq[:, j * 256:(j + 1) * 256])
        gproj(ren_ap[0], big_ap(wvr)[1], [v_post(4 + t) for t in range(4)])
        for j in range(8, NQTT):
            rope_tile(qwork[j], frq[:, j * 256:(j + 1) * 256],
                      fiq[:, j * 256:(j + 1) * 256])
        wo_big = wo_pool.tile([128, GH * DIM], F16, tag="wob", name="wo_big")
        nc.sync.dma_start(
            out=wo_big.rearrange("p (c n) -> p c n", c=GH),
            in_=dram["woTg"].rearrange("(c p) n -> p c n", p=128))

        # post-collective: batched stats, then q normalize + transpose
        # (emitted in j order so attention can start on the first tiles).
        # rk folds the 1/sqrt(hd) score scale:
        #   SCORE_SCALE / sqrt(ss/DIM + EPS) = 1 / sqrt(ss*HD/DIM + HD*EPS)
        rms_stats(rk_all, red[:, 0:NKVT], eps_hd[:], float(HD) / DIM)
        rms_stats(rq_all, red[:, NKVT:NST], eps_sb[:], 1.0 / DIM)

        for j in range(NQTT):
            nc.vector.tensor_scalar_mul(out=qwork[j][:], in0=qwork[j][:],
                                        scalar1=rq_all[:, j:j + 1])
        for j in range(NQTT):
            qch, jj = divmod(j, 8)
            dst = QT[qch].rearrange("p (d c) -> p d c", d=GH)[
                :, :, jj * 128:(jj + 1) * 128]
            transpose_tile(qwork[j], dst, ps_tr,
                           eng="act" if j % 2 else "dve")

    # ---------------- attention + o ----------------
    atp = ctx.enter_context(tc.tile_pool(name="atp", bufs=2))
    AT = [atp.tile([128, GH * 1024], F16, tag="at", name=f"AT{c}")
          for c in range(2)]
    expp = ctx.enter_context(tc.tile_pool(name="expp", bufs=5))
    l1p = ctx.enter_context(tc.tile_pool(name="l1p", bufs=5))
    l2p = ctx.enter_context(tc.tile_pool(name="l2p", bufs=3))
    accp = ctx.enter_context(tc.tile_pool(name="accp", bufs=2))
    stage_pool = ctx.enter_context(tc.tile_pool(name="stage", bufs=3))

    def attn_head(qch, h, ps_sc, ps_at, fill=None):
        """Software-pipelined flash-style head: scores run 2 kv-tiles ahead
        of exp/PV; denominator = pairwise tree of 4x-mode adds + one gpsimd
        partition reduce."""
        at_ps = [ps_at.tile([128, 512], F32, tag="at", name=f"at{qch}_{h}_{i}")
                 for i in range(2)]
        sc = [None] * NKVT
        ex = [None] * NKVT
        l1 = [None] * 4

        def emit_sc(kvt):
            s = ps_sc.tile([128, 1024], F32, tag="sc")
            for hf in range(2):
                nc.tensor.matmul(
                    s[:, hf * 512:(hf + 1) * 512],
                    KT[:, h * NKV + kvt * 128: h * NKV + (kvt + 1) * 128],
                    QT[qch][:, h * 1024 + hf * 512: h * 1024 + (hf + 1) * 512],
                    start=True, stop=True,
                )
            sc[kvt] = s

        emit_sc(0)
        if fill is not None:
            fill()
        emit_sc(1)
        for kvt in range(NKVT):
            e = expp.tile([128, 1024], EXDT, tag="exp")
            nc.scalar.activation(
                out=e[:], in_=sc[kvt][:],
                func=mybir.ActivationFunctionType.Exp,
                scale=rk_all[:, kvt:kvt + 1], bias=bias_sh[:],
            )
            ex[kvt] = e
            if kvt + 2 < NKVT:
                emit_sc(kvt + 2)
            for hf in range(2):
                nc.tensor.matmul(
                    at_ps[hf][:], Vg[kvt][:, h * 128:(h + 1) * 128],
                    e[:, hf * 512:(hf + 1) * 512],
                    start=(kvt == 0), stop=(kvt == NKVT - 1),
                )
            if kvt % 2 == 1:
                t = l1p.tile([128, 1024], EXDT, tag="l1")
                nc.vector.tensor_add(out=t[:], in0=ex[kvt - 1][:],
                                     in1=ex[kvt][:])
                l1[kvt // 2] = t
        l2a = l2p.tile([128, 1024], EXDT, tag="l2")
        l2b = l2p.tile([128, 1024], EXDT, tag="l2")
        nc.vector.tensor_add(out=l2a[:], in0=l1[0][:], in1=l1[1][:])
        nc.vector.tensor_add(out=l2b[:], in0=l1[2][:], in1=l1[3][:])
        acc = accp.tile([128, 1024], F32, tag="acc")
        nc.vector.tensor_add(out=acc[:], in0=l2a[:], in1=l2b[:])
        den = accp.tile([128, 1024], F32, tag="den")
        nc.gpsimd.partition_all_reduce(den[:], acc[:], channels=128,
                                       reduce_op=bass_isa.ReduceOp.add)
        nc.vector.reciprocal(out=den[:], in_=den[:])
        for hf in range(2):
            nc.vector.tensor_mul(
                out=AT[qch][:, h * 1024 + hf * 512: h * 1024 + (hf + 1) * 512],
                in0=at_ps[hf][:], in1=den[:, hf * 512:(hf + 1) * 512],
            )

    def o_tile(qch, tj, ot, ps_o, stage, copy_eng):
        """One [128,512] slice of the o projection for token tile tj of qch."""
        ps = ps_o.tile([128, 512], F32, tag="o")
        for hc in range(GH):
            nc.tensor.matmul(
                ps[:],
                AT[qch][:, hc * 1024 + tj * 128: hc * 1024 + (tj + 1) * 128],
                wo_big[:, hc * DIM + ot * 512: hc * DIM + (ot + 1) * 512],
                start=(hc == 0), stop=(hc == GH - 1),
            )
        if copy_eng == "act":
            nc.scalar.activation(out=stage[:, ot * 512:(ot + 1) * 512],
                                 in_=ps[:],
                                 func=mybir.ActivationFunctionType.Copy)
        else:
            nc.vector.tensor_copy(out=stage[:, ot * 512:(ot + 1) * 512],
                                  in_=ps[:])

    def o_token(qch, tj, ps_o):
        tt = qch * 8 + tj
        stage = stage_pool.tile([128, DIM], F16, tag="stage")
        for ot in range(4):
            o_tile(qch, tj, ot, ps_o, stage, "act" if ot % 2 else "dve")
        nc.sync.dma_start(
            out=dram["out"][tt * 128:(tt + 1) * 128, :], in_=stage[:])

    with (
        tc.tile_pool(name="ps_scA", bufs=2, space="PSUM") as ps_sc,
        tc.tile_pool(name="ps_atA", bufs=4, space="PSUM") as ps_at,
    ):
        for h in range(GH):
            attn_head(0, h, ps_sc, ps_at)

    with (
        tc.tile_pool(name="ps_scC", bufs=2, space="PSUM") as ps_sc,
        tc.tile_pool(name="ps_atC", bufs=2, space="PSUM") as ps_at,
        tc.tile_pool(name="ps_oC", bufs=2, space="PSUM") as ps_oC,
    ):
        def fill_for(h):
            if h == 0:
                return None

            def fill():
                for tj in (2 * (h - 1), 2 * (h - 1) + 1):
                    o_token(0, tj, ps_oC)
            return fill

        for h in range(GH):
            attn_head(1, h, ps_sc, ps_at, fill=fill_for(h))
        for tj in (6, 7):
            o_token(0, tj, ps_oC)

    with tc.tile_pool(name="ps_oD", bufs=6, space="PSUM") as ps_oD:
        for tj in range(8):
            o_token(1, tj, ps_oD)


_NC_CACHE = {}


def build_program():
    import os
    key = (os.environ.get("KERNEL_TIMING_REPS", "0"),)
    if key in _NC_CACHE:
        return _NC_CACHE[key]
    from contextlib import ExitStack

    nc = bacc.Bacc(
        "TRN2", target_bir_lowering=False, debug=False,
        enable_asserts=True, num_devices=N_CORES,
    )
    dram = {}
    specs = [
        ("xT", [DIM, NQT], F16),
        ("camT", [DIM, SC], F16),
        ("renT", [DIM, SR], F16),
        ("wqTg", [DIM, GD], F16),
        ("wkTg", [DIM, GD], F16),
        ("wvTg", [DIM, GD], F16),
        ("wkrTg", [DIM, GD], F16),
        ("wvrTg", [DIM, GD], F16),
        ("woTg", [GD, DIM], F16),
        ("frq", [NQT, GH * 64], F16),
        ("fiq", [NQT, GH * 64], F16),
        ("frc", [SC, GH * 64], F16),
        ("fic", [SC, GH * 64], F16),
        ("frr", [SR, GH * 64], F16),
        ("fir", [SR, GH * 64], F16),
    ]
    for name, shape, dt in specs:
        dram[name] = nc.dram_tensor(name, shape, dt, kind="ExternalInput").ap()
    dram["cc_in"] = nc.dram_tensor("cc_in", [NST * 128], F32, kind="Internal").ap()
    dram["cc_out"] = nc.dram_tensor("cc_out", [NST * 128], F32, kind="Internal").ap()
    dram["out"] = nc.dram_tensor("out", [NQT, DIM], F16, kind="ExternalOutput").ap()

    timing_reps = int(os.environ.get("KERNEL_TIMING_REPS", "0"))
    with tile.TileContext(nc) as tc:
        for _ in range(max(1, timing_reps)):
            with ExitStack() as ctx:
                _body_tp(ctx, tc, dram)
    nc.compile()
    _NC_CACHE[key] = nc
    return nc


def _expand_freqs(freqs, nh=GH):
    # freqs [s, 64, 2] -> fr, fi each [s, nh*64] (per-head repeat)
    fr = np.ascontiguousarray(
        np.broadcast_to(freqs[:, None, :, 0], (freqs.shape[0], nh, 64))
    ).reshape(freqs.shape[0], nh * 64)
    fi = np.ascontiguousarray(
        np.broadcast_to(freqs[:, None, :, 1], (freqs.shape[0], nh, 64))
    ).reshape(freqs.shape[0], nh * 64)
    return (np.ascontiguousarray(fr.astype(NPF16)),
            np.ascontiguousarray(fi.astype(NPF16)))


def _rope_perm():
    # de-interleave (re, im) pairs within each head's 128 dims:
    # new col h*128 + s*64 + i  <-  old col h*128 + 2*i + s
    perm = np.empty(GD, np.int64)
    for h in range(GH):
        for i in range(64):
            for s in range(2):
                perm[h * 128 + s * 64 + i] = h * 128 + 2 * i + s
    return perm


def make_in_maps_tp(x, cam_emb, render_emb, freqs_x, freqs_cam, freqs_render,
                    wq, bq, wk, bk, wv, bv, wkr, bkr, wvr, bvr, wo, bo, gq, gk):
    for b in (bq, bk, bv, bkr, bvr, bo):
        assert np.abs(np.asarray(b)).max() == 0.0, "nonzero bias unsupported"
    assert np.allclose(np.asarray(gq), 1.0) and np.allclose(np.asarray(gk), 1.0), \
        "non-unit rmsnorm gains unsupported"

    def wT(w):
        return np.asarray(w).T.astype(NPF16)

    wqT, wkT, wvT = wT(wq), wT(wk), wT(wv)
    wkrT, wvrT, woT = wT(wkr), wT(wvr), wT(wo)
    frq, fiq = _expand_freqs(np.asarray(freqs_x))
    frc, fic = _expand_freqs(np.asarray(freqs_cam))
    frr, fir = _expand_freqs(np.asarray(freqs_render))
    perm = _rope_perm()

    x = np.asarray(x)
    cam = np.asarray(cam_emb)
    ren = np.asarray(render_emb)
    xT = [np.ascontiguousarray(x[b].T.astype(NPF16)) for b in range(2)]
    camT = [np.ascontiguousarray(cam[b].T.astype(NPF16)) for b in range(2)]
    renT = [np.ascontiguousarray(ren[b].T.astype(NPF16)) for b in range(2)]
    in_maps = []
    for c in range(N_CORES):
        b, g = divmod(c, 4)
        gs = slice(g * GD, (g + 1) * GD)
        m = {
            "xT": xT[b], "camT": camT[b], "renT": renT[b],
            "wqTg": np.ascontiguousarray(wqT[:, gs][:, perm]),
            "wkTg": np.ascontiguousarray(wkT[:, gs][:, perm]),
            "wvTg": np.ascontiguousarray(wvT[:, gs]),
            "wkrTg": np.ascontiguousarray(wkrT[:, gs][:, perm]),
            "wvrTg": np.ascontiguousarray(wvrT[:, gs]),
            "woTg": np.ascontiguousarray(woT[gs, :]),
            "frq": frq, "fiq": fiq,
            "frc": frc, "fic": fic, "frr": frr, "fir": fir,
        }
        in_maps.append(m)
    return in_maps


def kernel(**inputs):
    nc = build_program()
    in_maps = make_in_maps_tp(**inputs)
    res = run_bass_kernel_spmd(nc, in_maps, core_ids=list(range(N_CORES)))
    x = np.asarray(inputs["x"])
    out = np.empty((x.shape[0], x.shape[1], DIM), dtype=np.float32)
    for b in range(2):
        acc = res.results[4 * b]["out"].astype(np.float32)
        for g in range(1, 4):
            acc = acc + res.results[4 * b + g]["out"].astype(np.float32)
        out[b] = acc
    out += np.asarray(inputs["bo"])[None, None, :]
    return out


def _make_timed_runner(nc, in_maps):
    """Reusable jitted SPMD callable with device-resident inputs."""
    import jax
    from jax.experimental.shard_map import shard_map
    from jax.sharding import Mesh, PartitionSpec, NamedSharding
    from concourse import bass2jax, mybir as mb

    bass2jax.install_neuronx_cc_hook()

    in_names, out_names, out_avals = [], [], []
    partition_name = nc.partition_id_tensor.name if nc.partition_id_tensor else None
    for alloc in nc.m.functions[0].allocations:
        if not isinstance(alloc, mb.MemoryLocationSet):
            continue
        name = alloc.memorylocations[0].name
        if alloc.kind == "ExternalInput":
            if name != partition_name:
                in_names.append(name)
        elif alloc.kind == "ExternalOutput":
            shape = tuple(alloc.tensor_shape)
            dtype = mb.dt.np(alloc.dtype)
            out_names.append(name)
            out_avals.append(jax.core.ShapedArray(shape, dtype))
    n_params = len(in_names)
    all_names = list(in_names) + list(out_names)
    if partition_name is not None:
        all_names.append(partition_name)

    def _body(*args):
        operands = list(args)
        if partition_name is not None:
            operands.append(bass2jax.partition_id_tensor())
        outs = bass2jax._bass_exec_p.bind(
            *operands,
            out_avals=tuple(out_avals),
            in_names=tuple(all_names),
            out_names=tuple(out_names),
            lowering_input_output_aliases=(),
            sim_require_finite=True,
            sim_require_nnan=True,
            nc=nc,
        )
        return tuple(outs)

    devices = jax.devices()[:N_CORES]
    mesh = Mesh(np.asarray(devices), ("core",))
    in_specs = (PartitionSpec("core"),) * (n_params + len(out_names))
    out_specs = (PartitionSpec("core"),) * len(out_names)
    sharded = jax.jit(
        shard_map(_body, mesh=mesh, in_specs=in_specs, out_specs=out_specs,
                  check_rep=False),
        keep_unused=True,
    )
    sharding = NamedSharding(mesh, PartitionSpec("core"))
    concat_in = [
        jax.device_put(
            np.concatenate([np.asarray(in_maps[c][nm]) for c in range(N_CORES)],
                           axis=0),
            sharding,
        )
        for nm in in_names
    ]
    for av in out_avals:
        concat_in.append(
            jax.device_put(
                np.zeros((N_CORES * av.shape[0], *av.shape[1:]), av.dtype), sharding
            )
        )
    return sharded, concat_in


def bench(inputs, iters=10):
    """Return per-execution device time in ns, amortized over `iters` runs."""
    import time
    import jax

    nc = build_program()
    in_maps = make_in_maps_tp(**inputs)
    fn, dev_in = _make_timed_runner(nc, in_maps)
    outs = fn(*dev_in)
    jax.block_until_ready(outs)
    t0 = time.perf_counter()
    for _ in range(iters):
        outs = fn(*dev_in)
    jax.block_until_ready(outs)
    dt = (time.perf_counter() - t0) / iters
    return dt * 1e9


# revision 34
# speedup vs baseline: 1.0842x; 1.0068x over previous
"""Trainium2 Bass kernel for nn_CrossAttentionCondition (tensor-parallel v4).

v4 over v3:
- float16 activations/weights everywhere (4x finer mantissa than bf16, same
  PE rate); exp output + V in f16 with the exp shifted by -12 so e^s fits
  f16 range (softmax is shift-invariant; EX_F16=False falls back to bf16
  attention with no shift).
- Batched DMAs: one strided dma_start per tensor (~40 total vs ~310), each
  [128, chunks*cols]; SP issue time drops ~150us.
- KT/QT/attnT merged into head-major [128, 4*1024] tiles so each PE
  transpose needs ONE strided copy instead of four.
- Attention inner loop software-pipelined (scores run 2 tiles ahead of
  exp/PV); softmax denominator via a bf16/f16 pairwise tree of
  scalar_tensor_tensor adds (4x DVE mode) + one gpsimd partition reduce,
  instead of 8 full-rate f32 vector adds.
- o(qch0) matmuls interleaved into the qch1 attention phase; all o outputs
  staged f16 and written with one dma per token tile.

Sharding: 8 cores = 2 batches x 4 head-groups (4 heads / 512 dims each).
Column-sharded q/k/v projections, row-sharded o with host-side gather-add.
RMSNorm sum-of-squares over the full 2048 dims -> ONE AllReduce of 24
per-token-tile stat columns (8 k + 16 q), kicked right after the q/k
projections; v projections and ropes run behind it. RoPE pairs are
de-interleaved host-side; q/k are roped unnormalized (rope commutes with
the per-token scale), rk folds the 1/sqrt(hd) into the exp scale, rq is
applied on the roped q tiles after the collective lands.
"""

import numpy as np
import ml_dtypes

import concourse.bass as bass
import concourse.tile as tile
from concourse import bacc, mybir, bass_isa
from concourse.bass_utils import run_bass_kernel_spmd
from concourse.masks import make_identity

F16 = mybir.dt.float16
BF16 = mybir.dt.bfloat16
F32 = mybir.dt.float32
NPF16 = np.float16
NPBF16 = ml_dtypes.bfloat16

# exp/V dtype: f16 with shifted exp if True, bf16 unshifted fallback.
EX_F16 = True
EXDT = F16 if EX_F16 else BF16
NPEXDT = NPF16 if EX_F16 else NPBF16
EXP_SHIFT = -12.0 if EX_F16 else 0.0

DIM = 2048
H = 16
HD = 128
SC = 512
SR = 512
NKV = SC + SR
EPS = 1e-6
N_CORES = 8

KC = DIM // 128   # 16 contraction chunks
GH = 4            # heads per core
GD = GH * HD      # 512
NQT = 2048        # q tokens per core (full batch)
RG = [[0, 1, 2, 3], [4, 5, 6, 7]]
NST = 8 + 16      # stat columns: 8 k tiles + 16 q tiles
NKVT = NKV // 128  # 8
NQTT = NQT // 128  # 16

MUL = mybir.AluOpType.mult
ADD = mybir.AluOpType.add


def _body_tp(ctx, tc, dram):
    nc = tc.nc

    const = ctx.enter_context(tc.tile_pool(name="const", bufs=1))
    ident = const.tile([128, 128], F16, tag="ident")
    make_identity(nc, ident)
    eps_sb = const.tile([128, 1], F32, tag="eps")
    nc.vector.memset(eps_sb, EPS)
    eps_hd = const.tile([128, 1], F32, tag="epshd")
    nc.vector.memset(eps_hd, float(HD * EPS))
    bias_sh = const.tile([128, 1], F32, tag="bsh")
    nc.vector.memset(bias_sh, EXP_SHIFT)
    # dummy sqrt pins the initial act table to sqrt_and_others (which also
    # contains Square/Copy), so the post-collective stats don't pay a table
    # load on the critical path.
    warm = const.tile([128, 1], F32, tag="warm")
    nc.scalar.activation(out=warm, in_=eps_sb[:],
                         func=mybir.ActivationFunctionType.Sqrt)

    # persistent activation tiles (head-major layouts)
    ktp = ctx.enter_context(tc.tile_pool(name="ktp", bufs=1))
    qtp = ctx.enter_context(tc.tile_pool(name="qtp", bufs=2))
    vp = ctx.enter_context(tc.tile_pool(name="vp", bufs=NKVT))
    KT = ktp.tile([128, GH * NKV], F16, tag="kt", name="KT")
    QT = [qtp.tile([128, GH * 1024], F16, tag="qt", name=f"QT{c}")
          for c in range(2)]
    Vg = [vp.tile([128, GD], EXDT, tag="v", name=f"Vg{i}") for i in range(NKVT)]

    ss_pool = ctx.enter_context(tc.tile_pool(name="statss", bufs=1))
    ss_all = ss_pool.tile([128, NST], F32, tag="ss", name="ss_all")
    red = ss_pool.tile([128, NST], F32, tag="red", name="red")
    rk_all = ss_pool.tile([128, NKVT], F32, tag="rk", name="rk_all")
    rq_all = ss_pool.tile([128, NQTT], F32, tag="rq", name="rq_all")
    stat_pool = ctx.enter_context(tc.tile_pool(name="stat", bufs=2))

    wo_pool = ctx.enter_context(tc.tile_pool(name="wo", bufs=1))

    def load_big(pool, name, nchunk, ncol, tag, col0=None, colw=None,
                 chunk0=0):
        """One strided DMA: dram rows [chunk0*128, (chunk0+nchunk)*128) (and
        optional col slice) -> [128, nchunk*ncol]."""
        t = pool.tile([128, nchunk * ncol], F16, tag=tag)
        src = dram[name][chunk0 * 128:(chunk0 + nchunk) * 128, :]
        if col0 is not None:
            src = src[:, col0:col0 + colw]
        nc.sync.dma_start(out=t.rearrange("p (c n) -> p c n", c=nchunk),
                          in_=src.rearrange("(c p) n -> p c n", p=128))
        return t

    def rms_stats(out_t, cols, bias, scale):
        std = stat_pool.tile([128, cols.shape[1]], F32, tag="std")
        nc.scalar.activation(
            out=std, in_=cols, func=mybir.ActivationFunctionType.Sqrt,
            bias=bias, scale=scale,
        )
        nc.vector.reciprocal(out=out_t, in_=std)

    def rope_tile(work_t, fr, fi):
        # in-place rope on the UNNORMALIZED tile; de-interleaved layout
        # (per head chunk [re(64) | im(64)]); all reads happen before writes.
        v4 = work_t.rearrange("p (h k i) -> p h k i", k=2, i=64)
        re, im = v4[:, :, 0, :], v4[:, :, 1, :]
        frv = fr.rearrange("p (h i) -> p h i", i=64)
        fiv = fi.rearrange("p (h i) -> p h i", i=64)
        t1 = rope_pool.tile([128, GH, 64], F16, tag="t1")
        t2 = rope_pool.tile([128, GH, 64], F16, tag="t2")
        t3 = rope_pool.tile([128, GH, 64], F16, tag="t3")
        t4 = rope_pool.tile([128, GH, 64], F16, tag="t4")
        nc.vector.tensor_mul(out=t1[:], in0=re, in1=frv)
        nc.vector.tensor_mul(out=t2[:], in0=im, in1=fiv)
        nc.vector.tensor_mul(out=t3[:], in0=re, in1=fiv)
        nc.vector.tensor_mul(out=t4[:], in0=im, in1=frv)
        nc.vector.tensor_sub(out=re, in0=t1[:], in1=t2[:])
        nc.vector.tensor_add(out=im, in0=t3[:], in1=t4[:])

    def transpose_tile(roped, dst3, ps_tr, eng="dve"):
        """PE-transpose a [128 tok, GD] tile into 4 head blocks and store via
        ONE strided copy into dst3 ([128, GH, 128] view of a big tile)."""
        pt = ps_tr.tile([128, GD], F16, tag="tr")
        for d in range(GH):
            nc.tensor.transpose(
                pt[:, d * 128:(d + 1) * 128], roped[:, d * 128:(d + 1) * 128],
                ident[:],
            )
        src3 = pt.rearrange("p (d c) -> p d c", c=128)
        if eng == "act":
            nc.scalar.activation(out=dst3, in_=src3,
                                 func=mybir.ActivationFunctionType.Copy)
        else:
            nc.vector.tensor_copy(out=dst3, in_=src3)

    # ---------------- projections, one CC, ropes, q transposes ------------
    with (
        tc.tile_pool(name="ps_proj", bufs=5, space="PSUM") as ps_proj,
        tc.tile_pool(name="ps_tr", bufs=2, space="PSUM") as ps_tr,
        tc.tile_pool(name="srcp", bufs=1) as src_pool,
        tc.tile_pool(name="xp", bufs=3) as x_pool,
        tc.tile_pool(name="kw", bufs=NKVT) as kw_pool,
        tc.tile_pool(name="qw", bufs=NQTT) as qw_pool,
        tc.tile_pool(name="rope", bufs=1) as rope_pool_,
        tc.tile_pool(name="freq", bufs=1) as freq_pool,
        tc.tile_pool(name="wbig", bufs=2) as w_pool,
    ):
        rope_pool = rope_pool_
        kwork = [kw_pool.tile([128, GD], F16, tag="kw", name=f"kw{i}")
                 for i in range(NKVT)]
        qwork = [qw_pool.tile([128, GD], F16, tag="qw", name=f"qw{i}")
                 for i in range(NQTT)]
        def big_ap(t):
            # accessor over one [128, KC*512] tile
            def src(kc, i):
                return t[:, kc * 512 + i * 128: kc * 512 + (i + 1) * 128]

            def wt(kc):
                return t[:, kc * 512:(kc + 1) * 512]
            return src, wt

        def half_ap(t0, t1):
            # accessor over two [128, 8*512] half tiles
            def src(kc, i):
                t = t0 if kc < 8 else t1
                c = kc % 8
                return t[:, c * 512 + i * 128: c * 512 + (i + 1) * 128]

            def wt(kc):
                t = t0 if kc < 8 else t1
                c = kc % 8
                return t[:, c * 512:(c + 1) * 512]
            return src, wt

        def gproj(src, wt, posts):
            for i, post in enumerate(posts):
                ps = ps_proj.tile([128, GD], F32, tag="proj")
                for kc in range(KC):
                    nc.tensor.matmul(
                        ps[:], src(kc, i), wt(kc),
                        start=(kc == 0), stop=(kc == KC - 1),
                    )
                post(ps)

        def gproj_first(src, wt, posts):
            # kc-split variant: runs chunks 0..7 for every output tile before
            # touching chunks 8..15, so compute starts after only the first
            # half of the src/weight DMAs has landed.
            ps_list = [ps_proj.tile([128, GD], F32, tag="proj",
                                    name=f"psf{i}")
                       for i in range(len(posts))]
            for half in (0, 1):
                for i in range(len(posts)):
                    for kc in range(half * 8, half * 8 + 8):
                        nc.tensor.matmul(
                            ps_list[i], src(kc, i), wt(kc),
                            start=(kc == 0), stop=(kc == KC - 1),
                        )
            for i, post in enumerate(posts):
                post(ps_list[i])

        def norm_post(work, col):
            def post(ps):
                nc.vector.tensor_copy(out=work[:], in_=ps[:])
                nc.scalar.activation(
                    out=ps[:], in_=ps[:],
                    func=mybir.ActivationFunctionType.Square,
                    accum_out=ss_all[:, col:col + 1],
                )
            return post

        def v_post(tt):
            def post(ps):
                nc.scalar.activation(
                    out=Vg[tt][:], in_=ps[:],
                    func=mybir.ActivationFunctionType.Copy,
                )
            return post

        def k_rope_transpose(tt):
            if tt < 4:
                fr, fi = frc, fic
                c0 = tt * 256
            else:
                fr, fi = frr, fir
                c0 = (tt - 4) * 256
            rope_tile(kwork[tt], fr[:, c0:c0 + 256], fi[:, c0:c0 + 256])
            dst = KT.rearrange("p (d kv) -> p d kv", d=GH)[
                :, :, tt * 128:(tt + 1) * 128]
            transpose_tile(kwork[tt], dst, ps_tr)

        # k projections (cam then render), stats into ss_all[:, 0..7].
        # cam/wk are loaded in halves and the first projection is kc-split
        # so PE starts after only the first half of the DMAs has landed.
        cam0 = load_big(src_pool, "camT", 8, 512, "cam0")
        wk0 = load_big(w_pool, "wkTg", 8, 512, "wh")
        cam1 = load_big(src_pool, "camT", 8, 512, "cam1", chunk0=8)
        wk1 = load_big(w_pool, "wkTg", 8, 512, "wh", chunk0=8)
        ren_src = load_big(src_pool, "renT", KC, 512, "ren")
        wkr = load_big(w_pool, "wkrTg", KC, 512, "w")
        cam_ap, _ = half_ap(cam0, cam1)
        _, wk_ap = half_ap(wk0, wk1)
        ren_ap, wkr_ap = big_ap(ren_src), big_ap(wkr)[1]
        gproj_first(cam_ap, wk_ap,
                    [norm_post(kwork[t], t) for t in range(4)])
        def load_x(ch):
            h0 = load_big(x_pool, "xT", 8, 512, "x",
                          col0=ch * 512, colw=512)
            h1 = load_big(x_pool, "xT", 8, 512, "x",
                          col0=ch * 512, colw=512, chunk0=8)
            return half_ap(h0, h1)[0]

        wq = load_big(w_pool, "wqTg", KC, 512, "w")
        x0 = load_x(0)
        gproj(ren_ap[0], wkr_ap,
              [norm_post(kwork[4 + t], 4 + t) for t in range(4)])

        # q projections, stats into ss_all[:, 8..23]; k ropes+transposes
        # interleave behind them; v/wo weight streams prefetch late.
        frc = load_big(freq_pool, "frc", 4, 256, "frc")
        fic = load_big(freq_pool, "fic", 4, 256, "fic")
        frr = load_big(freq_pool, "frr", 4, 256, "frr")
        fir = load_big(freq_pool, "fir", 4, 256, "fir")
        frq = load_big(freq_pool, "frq", NQTT, 256, "frq")
        fiq = load_big(freq_pool, "fiq", NQTT, 256, "fiq")
        wq_ap = big_ap(wq)
        wv = wvr = None
        for ch in range(4):
            xs_ap = x0 if ch == 0 else load_x(ch)
            gproj(xs_ap, wq_ap[1],
                  [norm_post(qwork[ch * 4 + i], 8 + ch * 4 + i)
                   for i in range(4)])
            if ch == 0:
                for tt in range(4):
                    k_rope_transpose(tt)
            elif ch == 1:
                for tt in range(4, NKVT):
                    k_rope_transpose(tt)
            elif ch == 2:
                wv = load_big(w_pool, "wvTg", KC, 512, "w")

        # ONE collective for all 24 stat columns. AllGather + local 3-add
        # reduce: the collective cores charge ~1.9x more for AllReduce than
        # AllGather, and the payload is tiny.
        import os
        _ablate = os.environ.get("KERNEL_ABLATE", "")
        nc.sync.dma_start(
            out=dram["cc_in"].rearrange("(p j) -> p j", p=128), in_=ss_all[:]
        )
        red4 = ss_pool.tile([128, 4, NST], F32, tag="red4", name="red4")
        if _ablate == "nocc":
            # timing ablation: skip the collective (numerics wrong)
            for g in range(4):
                nc.sync.dma_start(
                    out=red4[:, g, :],
                    in_=dram["cc_in"].rearrange("(p j) -> p j", p=128))
        else:
            nc.gpsimd.collective_compute(
                "AllGather", mybir.AluOpType.bypass,
                ins=[dram["cc_in"]], outs=[dram["cc_out"]],
                replica_groups=RG,
            )
            nc.sync.dma_start(
                out=red4[:],
                in_=dram["cc_out"].rearrange("(g p j) -> p g j", p=128, j=NST),
            )
        nc.vector.tensor_add(out=red4[:, 0, :], in0=red4[:, 0, :],
                             in1=red4[:, 1, :])
        nc.vector.tensor_add(out=red4[:, 2, :], in0=red4[:, 2, :],
                             in1=red4[:, 3, :])
        nc.vector.tensor_add(out=red[:], in0=red4[:, 0, :],
                             in1=red4[:, 2, :])

        # v projections and q ropes stream behind the collective
        gproj(cam_ap, big_ap(wv)[1], [v_post(t) for t in range(4)])
        wvr = load_big(w_pool, "wvrTg", KC, 512, "w")
        for j in range(8):
            rope_tile(qwork[j], frq[:, j * 256:(j + 1) * 256],
                      fiq[:, j * 256:(j + 1) * 256])
        gproj(ren_ap[0], big_ap(wvr)[1], [v_post(4 + t) for t in range(4)])
        for j in range(8, NQTT):
            rope_tile(qwork[j], frq[:, j * 256:(j + 1) * 256],
                      fiq[:, j * 256:(j + 1) * 256])
        wo_big = wo_pool.tile([128, GH * DIM], F16, tag="wob", name="wo_big")
        nc.sync.dma_start(
            out=wo_big.rearrange("p (c n) -> p c n", c=GH),
            in_=dram["woTg"].rearrange("(c p) n -> p c n", p=128))

        # post-collective: batched stats, then q normalize + transpose
        # (emitted in j order so attention can start on the first tiles).
        # rk folds the 1/sqrt(hd) score scale:
        #   SCORE_SCALE / sqrt(ss/DIM + EPS) = 1 / sqrt(ss*HD/DIM + HD*EPS)
        rms_stats(rk_all, red[:, 0:NKVT], eps_hd[:], float(HD) / DIM)
        rms_stats(rq_all, red[:, NKVT:NST], eps_sb[:], 1.0 / DIM)

        for j in range(NQTT):
            nc.vector.tensor_scalar_mul(out=qwork[j][:], in0=qwork[j][:],
                                        scalar1=rq_all[:, j:j + 1])
        for j in range(NQTT):
            qch, jj = divmod(j, 8)
            dst = QT[qch].rearrange("p (d c) -> p d c", d=GH)[
                :, :, jj * 128:(jj + 1) * 128]
            transpose_tile(qwork[j], dst, ps_tr,
                           eng="act" if j % 2 else "dve")

    # ---------------- attention + o ----------------
    atp = ctx.enter_context(tc.tile_pool(name="atp", bufs=2))
    AT = [atp.tile([128, GH * 1024], F16, tag="at", name=f"AT{c}")
          for c in range(2)]
    expp = ctx.enter_context(tc.tile_pool(name="expp", bufs=5))
    l1p = ctx.enter_context(tc.tile_pool(name="l1p", bufs=5))
    l2p = ctx.enter_context(tc.tile_pool(name="l2p", bufs=3))
    accp = ctx.enter_context(tc.tile_pool(name="accp", bufs=2))
    stage_pool = ctx.enter_context(tc.tile_pool(name="stage", bufs=3))

    def attn_head(qch, h, ps_sc, ps_at, fill=None):
        """Software-pipelined flash-style head: scores run 2 kv-tiles ahead
        of exp/PV; denominator = pairwise tree of 4x-mode adds + one gpsimd
        partition reduce."""
        at_ps = [ps_at.tile([128, 512], F32, tag="at", name=f"at{qch}_{h}_{i}")
                 for i in range(2)]
        sc = [None] * NKVT
        ex = [None] * NKVT
        l1 = [None] * 4

        def emit_sc(kvt):
            s = ps_sc.tile([128, 1024], F32, tag="sc")
            for hf in range(2):
                nc.tensor.matmul(
                    s[:, hf * 512:(hf + 1) * 512],
                    KT[:, h * NKV + kvt * 128: h * NKV + (kvt + 1) * 128],
                    QT[qch][:, h * 1024 + hf * 512: h * 1024 + (hf + 1) * 512],
                    start=True, stop=True,
                )
            sc[kvt] = s

        emit_sc(0)
        if fill is not None:
            fill()
        emit_sc(1)
        for kvt in range(NKVT):
            e = expp.tile([128, 1024], EXDT, tag="exp")
            nc.scalar.activation(
                out=e[:], in_=sc[kvt][:],
                func=mybir.ActivationFunctionType.Exp,
                scale=rk_all[:, kvt:kvt + 1], bias=bias_sh[:],
            )
            ex[kvt] = e
            if kvt + 2 < NKVT:
                emit_sc(kvt + 2)
            for hf in range(2):
                nc.tensor.matmul(
                    at_ps[hf][:], Vg[kvt][:, h * 128:(h + 1) * 128],
                    e[:, hf * 512:(hf + 1) * 512],
                    start=(kvt == 0), stop=(kvt == NKVT - 1),
                )
            if kvt % 2 == 1:
                t = l1p.tile([128, 1024], EXDT, tag="l1")
                nc.vector.tensor_add(out=t[:], in0=ex[kvt - 1][:],
                                     in1=ex[kvt][:])
                l1[kvt // 2] = t
        l2a = l2p.tile([128, 1024], EXDT, tag="l2")
        l2b = l2p.tile([128, 1024], EXDT, tag="l2")
        nc.vector.tensor_add(out=l2a[:], in0=l1[0][:], in1=l1[1][:])
        nc.vector.tensor_add(out=l2b[:], in0=l1[2][:], in1=l1[3][:])
        acc = accp.tile([128, 1024], F32, tag="acc")
        nc.vector.tensor_add(out=acc[:], in0=l2a[:], in1=l2b[:])
        den = accp.tile([128, 1024], F32, tag="den")
        nc.gpsimd.partition_all_reduce(den[:], acc[:], channels=128,
                                       reduce_op=bass_isa.ReduceOp.add)
        nc.vector.reciprocal(out=den[:], in_=den[:])
        for hf in range(2):
            nc.vector.tensor_mul(
                out=AT[qch][:, h * 1024 + hf * 512: h * 1024 + (hf + 1) * 512],
                in0=at_ps[hf][:], in1=den[:, hf * 512:(hf + 1) * 512],
            )

    def o_tile(qch, tj, ot, ps_o, stage, copy_eng):
        """One [128,512] slice of the o projection for token tile tj of qch."""
        ps = ps_o.tile([128, 512], F32, tag="o")
        for hc in range(GH):
            nc.tensor.matmul(
                ps[:],
                AT[qch][:, hc * 1024 + tj * 128: hc * 1024 + (tj + 1) * 128],
                wo_big[:, hc * DIM + ot * 512: hc * DIM + (ot + 1) * 512],
                start=(hc == 0), stop=(hc == GH - 1),
            )
        if copy_eng == "act":
            nc.scalar.activation(out=stage[:, ot * 512:(ot + 1) * 512],
                                 in_=ps[:],
                                 func=mybir.ActivationFunctionType.Copy)
        else:
            nc.vector.tensor_copy(out=stage[:, ot * 512:(ot + 1) * 512],
                                  in_=ps[:])

    def o_token(qch, tj, ps_o):
        tt = qch * 8 + tj
        stage = stage_pool.tile([128, DIM], F16, tag="stage")
        for ot in range(4):
            o_tile(qch, tj, ot, ps_o, stage, "act" if ot % 2 else "dve")
        nc.sync.dma_start(
            out=dram["out"][tt * 128:(tt + 1) * 128, :], in_=stage[:])

    with (
        tc.tile_pool(name="ps_scA", bufs=2, space="PSUM") as ps_sc,
        tc.tile_pool(name="ps_atA", bufs=4, space="PSUM") as ps_at,
    ):
        for h in range(GH):
            attn_head(0, h, ps_sc, ps_at)

    with (
        tc.tile_pool(name="ps_scC", bufs=2, space="PSUM") as ps_sc,
        tc.tile_pool(name="ps_atC", bufs=2, space="PSUM") as ps_at,
        tc.tile_pool(name="ps_oC", bufs=2, space="PSUM") as ps_oC,
    ):
        def fill_for(h):
            if h == 0:
                return None

            def fill():
                for tj in (2 * (h - 1), 2 * (h - 1) + 1):
                    o_token(0, tj, ps_oC)
            return fill

        for h in range(GH):
            attn_head(1, h, ps_sc, ps_at, fill=fill_for(h))
        for tj in (6, 7):
            o_token(0, tj, ps_oC)

    with tc.tile_pool(name="ps_oD", bufs=6, space="PSUM") as ps_oD:
        for tj in range(8):
            o_token(1, tj, ps_oD)


_NC_CACHE = {}


def build_program():
    import os
    key = (os.environ.get("KERNEL_TIMING_REPS", "0"),
           os.environ.get("KERNEL_ABLATE", ""))
    if key in _NC_CACHE:
        return _NC_CACHE[key]
    from contextlib import ExitStack

    nc = bacc.Bacc(
        "TRN2", target_bir_lowering=False, debug=False,
        enable_asserts=True, num_devices=N_CORES,
    )
    dram = {}
    specs = [
        ("xT", [DIM, NQT], F16),
        ("camT", [DIM, SC], F16),
        ("renT", [DIM, SR], F16),
        ("wqTg", [DIM, GD], F16),
        ("wkTg", [DIM, GD], F16),
        ("wvTg", [DIM, GD], F16),
        ("wkrTg", [DIM, GD], F16),
        ("wvrTg", [DIM, GD], F16),
        ("woTg", [GD, DIM], F16),
        ("frq", [NQT, GH * 64], F16),
        ("fiq", [NQT, GH * 64], F16),
        ("frc", [SC, GH * 64], F16),
        ("fic", [SC, GH * 64], F16),
        ("frr", [SR, GH * 64], F16),
        ("fir", [SR, GH * 64], F16),
    ]
    for name, shape, dt in specs:
        dram[name] = nc.dram_tensor(name, shape, dt, kind="ExternalInput").ap()
    dram["cc_in"] = nc.dram_tensor("cc_in", [NST * 128], F32, kind="Internal").ap()
    dram["cc_out"] = nc.dram_tensor("cc_out", [4 * NST * 128], F32,
                                    kind="Internal").ap()
    dram["out"] = nc.dram_tensor("out", [NQT, DIM], F16, kind="ExternalOutput").ap()

    timing_reps = int(os.environ.get("KERNEL_TIMING_REPS", "0"))
    with tile.TileContext(nc) as tc:
        for _ in range(max(1, timing_reps)):
            with ExitStack() as ctx:
                _body_tp(ctx, tc, dram)
    nc.compile()
    _NC_CACHE[key] = nc
    return nc


def _expand_freqs(freqs, nh=GH):
    # freqs [s, 64, 2] -> fr, fi each [s, nh*64] (per-head repeat)
    fr = np.ascontiguousarray(
        np.broadcast_to(freqs[:, None, :, 0], (freqs.shape[0], nh, 64))
    ).reshape(freqs.shape[0], nh * 64)
    fi = np.ascontiguousarray(
        np.broadcast_to(freqs[:, None, :, 1], (freqs.shape[0], nh, 64))
    ).reshape(freqs.shape[0], nh * 64)
    return (np.ascontiguousarray(fr.astype(NPF16)),
            np.ascontiguousarray(fi.astype(NPF16)))


def _rope_perm():
    # de-interleave (re, im) pairs within each head's 128 dims:
    # new col h*128 + s*64 + i  <-  old col h*128 + 2*i + s
    perm = np.empty(GD, np.int64)
    for h in range(GH):
        for i in range(64):
            for s in range(2):
                perm[h * 128 + s * 64 + i] = h * 128 + 2 * i + s
    return perm


def make_in_maps_tp(x, cam_emb, render_emb, freqs_x, freqs_cam, freqs_render,
                    wq, bq, wk, bk, wv, bv, wkr, bkr, wvr, bvr, wo, bo, gq, gk):
    for b in (bq, bk, bv, bkr, bvr, bo):
        assert np.abs(np.asarray(b)).max() == 0.0, "nonzero bias unsupported"
    assert np.allclose(np.asarray(gq), 1.0) and np.allclose(np.asarray(gk), 1.0), \
        "non-unit rmsnorm gains unsupported"

    def wT(w):
        return np.asarray(w).T.astype(NPF16)

    wqT, wkT, wvT = wT(wq), wT(wk), wT(wv)
    wkrT, wvrT, woT = wT(wkr), wT(wvr), wT(wo)
    frq, fiq = _expand_freqs(np.asarray(freqs_x))
    frc, fic = _expand_freqs(np.asarray(freqs_cam))
    frr, fir = _expand_freqs(np.asarray(freqs_render))
    perm = _rope_perm()

    x = np.asarray(x)
    cam = np.asarray(cam_emb)
    ren = np.asarray(render_emb)
    xT = [np.ascontiguousarray(x[b].T.astype(NPF16)) for b in range(2)]
    camT = [np.ascontiguousarray(cam[b].T.astype(NPF16)) for b in range(2)]
    renT = [np.ascontiguousarray(ren[b].T.astype(NPF16)) for b in range(2)]
    in_maps = []
    for c in range(N_CORES):
        b, g = divmod(c, 4)
        gs = slice(g * GD, (g + 1) * GD)
        m = {
            "xT": xT[b], "camT": camT[b], "renT": renT[b],
            "wqTg": np.ascontiguousarray(wqT[:, gs][:, perm]),
            "wkTg": np.ascontiguousarray(wkT[:, gs][:, perm]),
            "wvTg": np.ascontiguousarray(wvT[:, gs]),
            "wkrTg": np.ascontiguousarray(wkrT[:, gs][:, perm]),
            "wvrTg": np.ascontiguousarray(wvrT[:, gs]),
            "woTg": np.ascontiguousarray(woT[gs, :]),
            "frq": frq, "fiq": fiq,
            "frc": frc, "fic": fic, "frr": frr, "fir": fir,
        }
        in_maps.append(m)
    return in_maps


def kernel(**inputs):
    nc = build_program()
    in_maps = make_in_maps_tp(**inputs)
    res = run_bass_kernel_spmd(nc, in_maps, core_ids=list(range(N_CORES)))
    x = np.asarray(inputs["x"])
    out = np.empty((x.shape[0], x.shape[1], DIM), dtype=np.float32)
    for b in range(2):
        acc = res.results[4 * b]["out"].astype(np.float32)
        for g in range(1, 4):
            acc = acc + res.results[4 * b + g]["out"].astype(np.float32)
        out[b] = acc
    out += np.asarray(inputs["bo"])[None, None, :]
    return out


def _make_timed_runner(nc, in_maps):
    """Reusable jitted SPMD callable with device-resident inputs."""
    import jax
    from jax.experimental.shard_map import shard_map
    from jax.sharding import Mesh, PartitionSpec, NamedSharding
    from concourse import bass2jax, mybir as mb

    bass2jax.install_neuronx_cc_hook()

    in_names, out_names, out_avals = [], [], []
    partition_name = nc.partition_id_tensor.name if nc.partition_id_tensor else None
    for alloc in nc.m.functions[0].allocations:
        if not isinstance(alloc, mb.MemoryLocationSet):
            continue
        name = alloc.memorylocations[0].name
        if alloc.kind == "ExternalInput":
            if name != partition_name:
                in_names.append(name)
        elif alloc.kind == "ExternalOutput":
            shape = tuple(alloc.tensor_shape)
            dtype = mb.dt.np(alloc.dtype)
            out_names.append(name)
            out_avals.append(jax.core.ShapedArray(shape, dtype))
    n_params = len(in_names)
    all_names = list(in_names) + list(out_names)
    if partition_name is not None:
        all_names.append(partition_name)

    def _body(*args):
        operands = list(args)
        if partition_name is not None:
            operands.append(bass2jax.partition_id_tensor())
        outs = bass2jax._bass_exec_p.bind(
            *operands,
            out_avals=tuple(out_avals),
            in_names=tuple(all_names),
            out_names=tuple(out_names),
            lowering_input_output_aliases=(),
            sim_require_finite=True,
            sim_require_nnan=True,
            nc=nc,
        )
        return tuple(outs)

    devices = jax.devices()[:N_CORES]
    mesh = Mesh(np.asarray(devices), ("core",))
    in_specs = (PartitionSpec("core"),) * (n_params + len(out_names))
    out_specs = (PartitionSpec("core"),) * len(out_names)
    sharded = jax.jit(
        shard_map(_body, mesh=mesh, in_specs=in_specs, out_specs=out_specs,
                  check_rep=False),
        keep_unused=True,
    )
    sharding = NamedSharding(mesh, PartitionSpec("core"))
    concat_in = [
        jax.device_put(
            np.concatenate([np.asarray(in_maps[c][nm]) for c in range(N_CORES)],
                           axis=0),
            sharding,
        )
        for nm in in_names
    ]
    for av in out_avals:
        concat_in.append(
            jax.device_put(
                np.zeros((N_CORES * av.shape[0], *av.shape[1:]), av.dtype), sharding
            )
        )
    return sharded, concat_in


def bench(inputs, iters=10):
    """Return per-execution device time in ns, amortized over `iters` runs."""
    import time
    import jax

    nc = build_program()
    in_maps = make_in_maps_tp(**inputs)
    fn, dev_in = _make_timed_runner(nc, in_maps)
    outs = fn(*dev_in)
    jax.block_until_ready(outs)
    t0 = time.perf_counter()
    for _ in range(iters):
        outs = fn(*dev_in)
    jax.block_until_ready(outs)
    dt = (time.perf_counter() - t0) / iters
    return dt * 1e9


# revision 41
# speedup vs baseline: 1.1259x; 1.0384x over previous
"""Trainium2 Bass kernel for nn_CrossAttentionCondition (tensor-parallel v4).

v4 over v3:
- float16 activations/weights everywhere (4x finer mantissa than bf16, same
  PE rate); exp output + V in f16 with the exp shifted by -12 so e^s fits
  f16 range (softmax is shift-invariant; EX_F16=False falls back to bf16
  attention with no shift).
- Batched DMAs: one strided dma_start per tensor (~40 total vs ~310), each
  [128, chunks*cols]; SP issue time drops ~150us.
- KT/QT/attnT merged into head-major [128, 4*1024] tiles so each PE
  transpose needs ONE strided copy instead of four.
- Attention inner loop software-pipelined (scores run 2 tiles ahead of
  exp/PV); softmax denominator via a bf16/f16 pairwise tree of
  scalar_tensor_tensor adds (4x DVE mode) + one gpsimd partition reduce,
  instead of 8 full-rate f32 vector adds.
- o(qch0) matmuls interleaved into the qch1 attention phase; all o outputs
  staged f16 and written with one dma per token tile.

Sharding: 8 cores = 2 batches x 4 head-groups (4 heads / 512 dims each).
Column-sharded q/k/v projections, row-sharded o with host-side gather-add.
RMSNorm sum-of-squares over the full 2048 dims -> ONE AllReduce of 24
per-token-tile stat columns (8 k + 16 q), kicked right after the q/k
projections; v projections and ropes run behind it. RoPE pairs are
de-interleaved host-side; q/k are roped unnormalized (rope commutes with
the per-token scale), rk folds the 1/sqrt(hd) into the exp scale, rq is
applied on the roped q tiles after the collective lands.
"""

import numpy as np
import ml_dtypes

import concourse.bass as bass
import concourse.tile as tile
from concourse import bacc, mybir, bass_isa
from concourse.bass_utils import run_bass_kernel_spmd
from concourse.masks import make_identity

F16 = mybir.dt.float16
BF16 = mybir.dt.bfloat16
F32 = mybir.dt.float32
NPF16 = np.float16
NPBF16 = ml_dtypes.bfloat16

# exp/V dtype: f16 with shifted exp if True, bf16 unshifted fallback.
EX_F16 = True
EXDT = F16 if EX_F16 else BF16
NPEXDT = NPF16 if EX_F16 else NPBF16
EXP_SHIFT = -12.0 if EX_F16 else 0.0

DIM = 2048
H = 16
HD = 128
SC = 512
SR = 512
NKV = SC + SR
EPS = 1e-6
N_CORES = 8

KC = DIM // 128   # 16 contraction chunks
GH = 4            # heads per core
GD = GH * HD      # 512
NQT = 2048        # q tokens per core (full batch)
RG = [[0, 1, 2, 3], [4, 5, 6, 7]]
NST = 8 + 16      # stat columns: 8 k tiles + 16 q tiles
NKVT = NKV // 128  # 8
NQTT = NQT // 128  # 16

MUL = mybir.AluOpType.mult
ADD = mybir.AluOpType.add


def _body_tp(ctx, tc, dram):
    nc = tc.nc

    const = ctx.enter_context(tc.tile_pool(name="const", bufs=1))
    ident = const.tile([128, 128], F16, tag="ident")
    make_identity(nc, ident)
    eps_sb = const.tile([128, 1], F32, tag="eps")
    nc.vector.memset(eps_sb, EPS)
    eps_hd = const.tile([128, 1], F32, tag="epshd")
    nc.vector.memset(eps_hd, float(HD * EPS))
    bias_sh = const.tile([128, 1], F32, tag="bsh")
    nc.vector.memset(bias_sh, EXP_SHIFT)
    # dummy sqrt pins the initial act table to sqrt_and_others (which also
    # contains Square/Copy), so the post-collective stats don't pay a table
    # load on the critical path.
    warm = const.tile([128, 1], F32, tag="warm")
    nc.scalar.activation(out=warm, in_=eps_sb[:],
                         func=mybir.ActivationFunctionType.Sqrt)

    # persistent activation tiles (head-major layouts)
    ktp = ctx.enter_context(tc.tile_pool(name="ktp", bufs=1))
    qtp = ctx.enter_context(tc.tile_pool(name="qtp", bufs=2))
    vp = ctx.enter_context(tc.tile_pool(name="vp", bufs=NKVT))
    KT = ktp.tile([128, GH * NKV], F16, tag="kt", name="KT")
    QT = [qtp.tile([128, GH * 1024], F16, tag="qt", name=f"QT{c}")
          for c in range(2)]
    Vg = [vp.tile([128, GD], EXDT, tag="v", name=f"Vg{i}") for i in range(NKVT)]

    ss_pool = ctx.enter_context(tc.tile_pool(name="statss", bufs=1))
    ss_all = ss_pool.tile([128, NST], F32, tag="ss", name="ss_all")
    red = ss_pool.tile([128, NST], F32, tag="red", name="red")
    rk_all = ss_pool.tile([128, NKVT], F32, tag="rk", name="rk_all")
    rq_all = ss_pool.tile([128, NQTT], F32, tag="rq", name="rq_all")
    stat_pool = ctx.enter_context(tc.tile_pool(name="stat", bufs=2))

    wo_pool = ctx.enter_context(tc.tile_pool(name="wo", bufs=1))

    def load_big(pool, name, nchunk, ncol, tag, col0=None, colw=None,
                 chunk0=0):
        """One strided DMA: dram rows [chunk0*128, (chunk0+nchunk)*128) (and
        optional col slice) -> [128, nchunk*ncol]."""
        t = pool.tile([128, nchunk * ncol], F16, tag=tag)
        src = dram[name][chunk0 * 128:(chunk0 + nchunk) * 128, :]
        if col0 is not None:
            src = src[:, col0:col0 + colw]
        nc.sync.dma_start(out=t.rearrange("p (c n) -> p c n", c=nchunk),
                          in_=src.rearrange("(c p) n -> p c n", p=128))
        return t

    def rms_stats(out_t, cols, bias, scale):
        std = stat_pool.tile([128, cols.shape[1]], F32, tag="std")
        nc.scalar.activation(
            out=std, in_=cols, func=mybir.ActivationFunctionType.Sqrt,
            bias=bias, scale=scale,
        )
        nc.vector.reciprocal(out=out_t, in_=std)

    def rope_tile(work_t, fr, fi):
        # in-place rope on the UNNORMALIZED tile; de-interleaved layout
        # (per head chunk [re(64) | im(64)]); all reads happen before writes.
        v4 = work_t.rearrange("p (h k i) -> p h k i", k=2, i=64)
        re, im = v4[:, :, 0, :], v4[:, :, 1, :]
        frv = fr.rearrange("p (h i) -> p h i", i=64)
        fiv = fi.rearrange("p (h i) -> p h i", i=64)
        t1 = rope_pool.tile([128, GH, 64], F16, tag="t1")
        t2 = rope_pool.tile([128, GH, 64], F16, tag="t2")
        t3 = rope_pool.tile([128, GH, 64], F16, tag="t3")
        t4 = rope_pool.tile([128, GH, 64], F16, tag="t4")
        nc.vector.tensor_mul(out=t1[:], in0=re, in1=frv)
        nc.vector.tensor_mul(out=t2[:], in0=im, in1=fiv)
        nc.vector.tensor_mul(out=t3[:], in0=re, in1=fiv)
        nc.vector.tensor_mul(out=t4[:], in0=im, in1=frv)
        nc.vector.tensor_sub(out=re, in0=t1[:], in1=t2[:])
        nc.vector.tensor_add(out=im, in0=t3[:], in1=t4[:])

    def transpose_tile(roped, dst3, ps_tr, eng="dve"):
        """PE-transpose a [128 tok, GD] tile into 4 head blocks and store via
        ONE strided copy into dst3 ([128, GH, 128] view of a big tile)."""
        pt = ps_tr.tile([128, GD], F16, tag="tr")
        for d in range(GH):
            nc.tensor.transpose(
                pt[:, d * 128:(d + 1) * 128], roped[:, d * 128:(d + 1) * 128],
                ident[:],
            )
        src3 = pt.rearrange("p (d c) -> p d c", c=128)
        if eng == "act":
            nc.scalar.activation(out=dst3, in_=src3,
                                 func=mybir.ActivationFunctionType.Copy)
        else:
            nc.vector.tensor_copy(out=dst3, in_=src3)

    # ---------------- projections, one CC, ropes, q transposes ------------
    with (
        tc.tile_pool(name="ps_warm", bufs=1, space="PSUM") as ps_warm,
        tc.tile_pool(name="ps_proj", bufs=5, space="PSUM") as ps_proj,
        tc.tile_pool(name="ps_tr", bufs=2, space="PSUM") as ps_tr,
        tc.tile_pool(name="srcp", bufs=1) as src_pool,
        tc.tile_pool(name="xp", bufs=3) as x_pool,
        tc.tile_pool(name="kw", bufs=NKVT) as kw_pool,
        tc.tile_pool(name="qw", bufs=NQTT) as qw_pool,
        tc.tile_pool(name="rope", bufs=1) as rope_pool_,
        tc.tile_pool(name="freq", bufs=1) as freq_pool,
        tc.tile_pool(name="wbig", bufs=2) as w_pool,
    ):
        rope_pool = rope_pool_
        kwork = [kw_pool.tile([128, GD], F16, tag="kw", name=f"kw{i}")
                 for i in range(NKVT)]
        qwork = [qw_pool.tile([128, GD], F16, tag="qw", name=f"qw{i}")
                 for i in range(NQTT)]
        def big_ap(t):
            # accessor over one [128, KC*512] tile
            def src(kc, i):
                return t[:, kc * 512 + i * 128: kc * 512 + (i + 1) * 128]

            def wt(kc):
                return t[:, kc * 512:(kc + 1) * 512]
            return src, wt

        def half_ap(t0, t1):
            # accessor over two [128, 8*512] half tiles
            def src(kc, i):
                t = t0 if kc < 8 else t1
                c = kc % 8
                return t[:, c * 512 + i * 128: c * 512 + (i + 1) * 128]

            def wt(kc):
                t = t0 if kc < 8 else t1
                c = kc % 8
                return t[:, c * 512:(c + 1) * 512]
            return src, wt

        def gproj(src, wt, posts):
            for i, post in enumerate(posts):
                ps = ps_proj.tile([128, GD], F32, tag="proj")
                for kc in range(KC):
                    nc.tensor.matmul(
                        ps[:], src(kc, i), wt(kc),
                        start=(kc == 0), stop=(kc == KC - 1),
                    )
                post(ps)

        def gproj_first(src, wt, posts):
            # kc-split variant: runs chunks 0..7 for every output tile before
            # touching chunks 8..15, so compute starts after only the first
            # half of the src/weight DMAs has landed.
            ps_list = [ps_proj.tile([128, GD], F32, tag="proj",
                                    name=f"psf{i}")
                       for i in range(len(posts))]
            for half in (0, 1):
                for i in range(len(posts)):
                    for kc in range(half * 8, half * 8 + 8):
                        nc.tensor.matmul(
                            ps_list[i], src(kc, i), wt(kc),
                            start=(kc == 0), stop=(kc == KC - 1),
                        )
            for i, post in enumerate(posts):
                post(ps_list[i])

        def norm_post(work, col):
            def post(ps):
                nc.vector.tensor_copy(out=work[:], in_=ps[:])
                nc.scalar.activation(
                    out=ps[:], in_=ps[:],
                    func=mybir.ActivationFunctionType.Square,
                    accum_out=ss_all[:, col:col + 1],
                )
            return post

        def v_post(tt):
            def post(ps):
                nc.scalar.activation(
                    out=Vg[tt][:], in_=ps[:],
                    func=mybir.ActivationFunctionType.Copy,
                )
            return post

        def k_rope_transpose(tt):
            if tt < 4:
                fr, fi = frc, fic
                c0 = tt * 256
            else:
                fr, fi = frr, fir
                c0 = (tt - 4) * 256
            rope_tile(kwork[tt], fr[:, c0:c0 + 256], fi[:, c0:c0 + 256])
            dst = KT.rearrange("p (d kv) -> p d kv", d=GH)[
                :, :, tt * 128:(tt + 1) * 128]
            transpose_tile(kwork[tt], dst, ps_tr)

        # PE p-state warm-up: ~3us of dummy matmuls on the identity while the
        # first input DMAs stream in, so real matmuls start at full clock.
        warm_ps = ps_warm.tile([128, 128], F32, tag="wps", name="warm_ps")
        for wi in range(18):
            nc.tensor.matmul(warm_ps[:], ident[:], ident[:],
                             start=(wi == 0), stop=(wi == 17))

        # k projections (cam then render), stats into ss_all[:, 0..7].
        # cam/wk are loaded in halves and the first projection is kc-split
        # so PE starts after only the first half of the DMAs has landed.
        cam0 = load_big(src_pool, "camT", 8, 512, "cam0")
        wk0 = load_big(w_pool, "wkTg", 8, 512, "wh")
        cam1 = load_big(src_pool, "camT", 8, 512, "cam1", chunk0=8)
        wk1 = load_big(w_pool, "wkTg", 8, 512, "wh", chunk0=8)
        ren_src = load_big(src_pool, "renT", KC, 512, "ren")
        wkr = load_big(w_pool, "wkrTg", KC, 512, "w")
        cam_ap, _ = half_ap(cam0, cam1)
        _, wk_ap = half_ap(wk0, wk1)
        ren_ap, wkr_ap = big_ap(ren_src), big_ap(wkr)[1]
        gproj_first(cam_ap, wk_ap,
                    [norm_post(kwork[t], t) for t in range(4)])
        def load_x(ch):
            h0 = load_big(x_pool, "xT", 8, 512, "x",
                          col0=ch * 512, colw=512)
            h1 = load_big(x_pool, "xT", 8, 512, "x",
                          col0=ch * 512, colw=512, chunk0=8)
            return half_ap(h0, h1)[0]

        wq = load_big(w_pool, "wqTg", KC, 512, "w")
        x0 = load_x(0)
        gproj(ren_ap[0], wkr_ap,
              [norm_post(kwork[4 + t], 4 + t) for t in range(4)])

        # q projections, stats into ss_all[:, 8..23]; k ropes+transposes
        # interleave behind them; v/wo weight streams prefetch late.
        frc = load_big(freq_pool, "frc", 4, 256, "frc")
        fic = load_big(freq_pool, "fic", 4, 256, "fic")
        frr = load_big(freq_pool, "frr", 4, 256, "frr")
        fir = load_big(freq_pool, "fir", 4, 256, "fir")
        frq = load_big(freq_pool, "frq", NQTT, 256, "frq")
        fiq = load_big(freq_pool, "fiq", NQTT, 256, "fiq")
        wq_ap = big_ap(wq)
        wv = wvr = None
        for ch in range(4):
            xs_ap = x0 if ch == 0 else load_x(ch)
            gproj(xs_ap, wq_ap[1],
                  [norm_post(qwork[ch * 4 + i], 8 + ch * 4 + i)
                   for i in range(4)])
            if ch == 0:
                for tt in range(4):
                    k_rope_transpose(tt)
            elif ch == 1:
                for tt in range(4, NKVT):
                    k_rope_transpose(tt)
            elif ch == 2:
                wv = load_big(w_pool, "wvTg", KC, 512, "w")

        # ONE collective for all 24 stat columns. AllGather + local 3-add
        # reduce: the collective cores charge ~1.9x more for AllReduce than
        # AllGather, and the payload is tiny.
        import os
        _ablate = os.environ.get("KERNEL_ABLATE", "")
        nc.sync.dma_start(
            out=dram["cc_in"].rearrange("(p j) -> p j", p=128), in_=ss_all[:]
        )
        red4 = ss_pool.tile([128, 4, NST], F32, tag="red4", name="red4")
        if _ablate == "nocc":
            # timing ablation: skip the collective (numerics wrong)
            for g in range(4):
                nc.sync.dma_start(
                    out=red4[:, g, :],
                    in_=dram["cc_in"].rearrange("(p j) -> p j", p=128))
        else:
            nc.gpsimd.collective_compute(
                "AllGather", mybir.AluOpType.bypass,
                ins=[dram["cc_in"]], outs=[dram["cc_out"]],
                replica_groups=RG,
            )
            nc.sync.dma_start(
                out=red4[:],
                in_=dram["cc_out"].rearrange("(g p j) -> p g j", p=128, j=NST),
            )
        nc.vector.tensor_add(out=red4[:, 0, :], in0=red4[:, 0, :],
                             in1=red4[:, 1, :])
        nc.vector.tensor_add(out=red4[:, 2, :], in0=red4[:, 2, :],
                             in1=red4[:, 3, :])
        nc.vector.tensor_add(out=red[:], in0=red4[:, 0, :],
                             in1=red4[:, 2, :])

        # v projections and q ropes stream behind the collective
        gproj(cam_ap, big_ap(wv)[1], [v_post(t) for t in range(4)])
        wvr = load_big(w_pool, "wvrTg", KC, 512, "w")
        for j in range(8):
            rope_tile(qwork[j], frq[:, j * 256:(j + 1) * 256],
                      fiq[:, j * 256:(j + 1) * 256])
        gproj(ren_ap[0], big_ap(wvr)[1], [v_post(4 + t) for t in range(4)])
        for j in range(8, NQTT):
            rope_tile(qwork[j], frq[:, j * 256:(j + 1) * 256],
                      fiq[:, j * 256:(j + 1) * 256])
        wo_big = wo_pool.tile([128, GH * DIM], F16, tag="wob", name="wo_big")
        nc.sync.dma_start(
            out=wo_big.rearrange("p (c n) -> p c n", c=GH),
            in_=dram["woTg"].rearrange("(c p) n -> p c n", p=128))

        # post-collective: batched stats, then q normalize + transpose
        # (emitted in j order so attention can start on the first tiles).
        # rk folds the 1/sqrt(hd) score scale:
        #   SCORE_SCALE / sqrt(ss/DIM + EPS) = 1 / sqrt(ss*HD/DIM + HD*EPS)
        rms_stats(rk_all, red[:, 0:NKVT], eps_hd[:], float(HD) / DIM)
        rms_stats(rq_all, red[:, NKVT:NST], eps_sb[:], 1.0 / DIM)

        for j in range(NQTT):
            nc.vector.tensor_scalar_mul(out=qwork[j][:], in0=qwork[j][:],
                                        scalar1=rq_all[:, j:j + 1])
        for j in range(NQTT):
            qch, jj = divmod(j, 8)
            dst = QT[qch].rearrange("p (d c) -> p d c", d=GH)[
                :, :, jj * 128:(jj + 1) * 128]
            transpose_tile(qwork[j], dst, ps_tr,
                           eng="act" if j % 2 else "dve")

    if _ablate == "proj":
        return

    # ---------------- attention + o ----------------
    atp = ctx.enter_context(tc.tile_pool(name="atp", bufs=2))
    AT = [atp.tile([128, GH * 1024], F16, tag="at", name=f"AT{c}")
          for c in range(2)]
    expp = ctx.enter_context(tc.tile_pool(name="expp", bufs=5))
    l1p = ctx.enter_context(tc.tile_pool(name="l1p", bufs=5))
    l2p = ctx.enter_context(tc.tile_pool(name="l2p", bufs=3))
    accp = ctx.enter_context(tc.tile_pool(name="accp", bufs=2))
    stage_pool = ctx.enter_context(tc.tile_pool(name="stage", bufs=6))

    def attn_head(qch, h, ps_sc, ps_at, fill=None):
        """Software-pipelined flash-style head: scores run 2 kv-tiles ahead
        of exp/PV; denominator = pairwise tree of 4x-mode adds + one gpsimd
        partition reduce."""
        at_ps = [ps_at.tile([128, 512], F32, tag="at", name=f"at{qch}_{h}_{i}")
                 for i in range(2)]
        sc = [None] * NKVT
        ex = [None] * NKVT
        l1 = [None] * 4

        def emit_sc(kvt):
            s = ps_sc.tile([128, 1024], F32, tag="sc")
            for hf in range(2):
                nc.tensor.matmul(
                    s[:, hf * 512:(hf + 1) * 512],
                    KT[:, h * NKV + kvt * 128: h * NKV + (kvt + 1) * 128],
                    QT[qch][:, h * 1024 + hf * 512: h * 1024 + (hf + 1) * 512],
                    start=True, stop=True,
                )
            sc[kvt] = s

        emit_sc(0)
        if fill is not None:
            fill()
        emit_sc(1)
        for kvt in range(NKVT):
            e = expp.tile([128, 1024], EXDT, tag="exp")
            nc.scalar.activation(
                out=e[:], in_=sc[kvt][:],
                func=mybir.ActivationFunctionType.Exp,
                scale=rk_all[:, kvt:kvt + 1], bias=bias_sh[:],
            )
            ex[kvt] = e
            if kvt + 2 < NKVT:
                emit_sc(kvt + 2)
            for hf in range(2):
                nc.tensor.matmul(
                    at_ps[hf][:], Vg[kvt][:, h * 128:(h + 1) * 128],
                    e[:, hf * 512:(hf + 1) * 512],
                    start=(kvt == 0), stop=(kvt == NKVT - 1),
                )
            if kvt % 2 == 1:
                t = l1p.tile([128, 1024], EXDT, tag="l1")
                nc.vector.tensor_add(out=t[:], in0=ex[kvt - 1][:],
                                     in1=ex[kvt][:])
                l1[kvt // 2] = t
        l2a = l2p.tile([128, 1024], EXDT, tag="l2")
        l2b = l2p.tile([128, 1024], EXDT, tag="l2")
        nc.vector.tensor_add(out=l2a[:], in0=l1[0][:], in1=l1[1][:])
        nc.vector.tensor_add(out=l2b[:], in0=l1[2][:], in1=l1[3][:])
        acc = accp.tile([128, 1024], F32, tag="acc")
        nc.vector.tensor_add(out=acc[:], in0=l2a[:], in1=l2b[:])
        den = accp.tile([128, 1024], F32, tag="den")
        nc.gpsimd.partition_all_reduce(den[:], acc[:], channels=128,
                                       reduce_op=bass_isa.ReduceOp.add)
        nc.vector.reciprocal(out=den[:], in_=den[:])
        for hf in range(2):
            nc.vector.tensor_mul(
                out=AT[qch][:, h * 1024 + hf * 512: h * 1024 + (hf + 1) * 512],
                in0=at_ps[hf][:], in1=den[:, hf * 512:(hf + 1) * 512],
            )

    def o_tile(qch, tj, ot, ps_o, stage, copy_eng):
        """One [128,512] slice of the o projection for token tile tj of qch."""
        ps = ps_o.tile([128, 512], F32, tag="o")
        for hc in range(GH):
            nc.tensor.matmul(
                ps[:],
                AT[qch][:, hc * 1024 + tj * 128: hc * 1024 + (tj + 1) * 128],
                wo_big[:, hc * DIM + ot * 512: hc * DIM + (ot + 1) * 512],
                start=(hc == 0), stop=(hc == GH - 1),
            )
        if copy_eng == "act":
            nc.scalar.activation(out=stage[:, ot * 512:(ot + 1) * 512],
                                 in_=ps[:],
                                 func=mybir.ActivationFunctionType.Copy)
        else:
            nc.vector.tensor_copy(out=stage[:, ot * 512:(ot + 1) * 512],
                                  in_=ps[:])

    def o_token(qch, tj, ps_o):
        tt = qch * 8 + tj
        stage = stage_pool.tile([128, DIM], F16, tag="stage")
        for ot in range(4):
            o_tile(qch, tj, ot, ps_o, stage, "act" if ot % 2 else "dve")
        if _ablate == "noout":
            return
        # issue output stores from the ACT HWDGE queue: the SP queue carries
        # all input loads, and sharing it serializes stores behind them
        # (~74us measured on hw)
        if _ablate == "outpool":
            eng = nc.gpsimd
        elif _ablate == "outsp":
            eng = nc.sync
        else:
            eng = nc.scalar
        eng.dma_start(
            out=dram["out"][tt * 128:(tt + 1) * 128, :], in_=stage[:])

    with (
        tc.tile_pool(name="ps_scA", bufs=2, space="PSUM") as ps_sc,
        tc.tile_pool(name="ps_atA", bufs=4, space="PSUM") as ps_at,
    ):
        for h in range(GH):
            attn_head(0, h, ps_sc, ps_at)

    with (
        tc.tile_pool(name="ps_scC", bufs=2, space="PSUM") as ps_sc,
        tc.tile_pool(name="ps_atC", bufs=2, space="PSUM") as ps_at,
        tc.tile_pool(name="ps_oC", bufs=2, space="PSUM") as ps_oC,
    ):
        def fill_for(h):
            if h == 0:
                return None

            def fill():
                for tj in (2 * (h - 1), 2 * (h - 1) + 1):
                    o_token(0, tj, ps_oC)
            return fill

        for h in range(GH):
            attn_head(1, h, ps_sc, ps_at, fill=fill_for(h))
        for tj in (6, 7):
            o_token(0, tj, ps_oC)

    with tc.tile_pool(name="ps_oD", bufs=6, space="PSUM") as ps_oD:
        for tj in range(8):
            o_token(1, tj, ps_oD)


_NC_CACHE = {}


def build_program():
    import os
    key = (os.environ.get("KERNEL_TIMING_REPS", "0"),
           os.environ.get("KERNEL_ABLATE", ""))
    if key in _NC_CACHE:
        return _NC_CACHE[key]
    from contextlib import ExitStack

    nc = bacc.Bacc(
        "TRN2", target_bir_lowering=False, debug=False,
        enable_asserts=True, num_devices=N_CORES,
    )
    dram = {}
    specs = [
        ("xT", [DIM, NQT], F16),
        ("camT", [DIM, SC], F16),
        ("renT", [DIM, SR], F16),
        ("wqTg", [DIM, GD], F16),
        ("wkTg", [DIM, GD], F16),
        ("wvTg", [DIM, GD], F16),
        ("wkrTg", [DIM, GD], F16),
        ("wvrTg", [DIM, GD], F16),
        ("woTg", [GD, DIM], F16),
        ("frq", [NQT, GH * 64], F16),
        ("fiq", [NQT, GH * 64], F16),
        ("frc", [SC, GH * 64], F16),
        ("fic", [SC, GH * 64], F16),
        ("frr", [SR, GH * 64], F16),
        ("fir", [SR, GH * 64], F16),
    ]
    for name, shape, dt in specs:
        dram[name] = nc.dram_tensor(name, shape, dt, kind="ExternalInput").ap()
    dram["cc_in"] = nc.dram_tensor("cc_in", [NST * 128], F32, kind="Internal").ap()
    dram["cc_out"] = nc.dram_tensor("cc_out", [4 * NST * 128], F32,
                                    kind="Internal").ap()
    dram["out"] = nc.dram_tensor("out", [NQT, DIM], F16, kind="ExternalOutput").ap()

    timing_reps = int(os.environ.get("KERNEL_TIMING_REPS", "0"))
    with tile.TileContext(nc) as tc:
        for _ in range(max(1, timing_reps)):
            with ExitStack() as ctx:
                _body_tp(ctx, tc, dram)
    nc.compile()
    _NC_CACHE[key] = nc
    return nc


def _expand_freqs(freqs, nh=GH):
    # freqs [s, 64, 2] -> fr, fi each [s, nh*64] (per-head repeat)
    fr = np.ascontiguousarray(
        np.broadcast_to(freqs[:, None, :, 0], (freqs.shape[0], nh, 64))
    ).reshape(freqs.shape[0], nh * 64)
    fi = np.ascontiguousarray(
        np.broadcast_to(freqs[:, None, :, 1], (freqs.shape[0], nh, 64))
    ).reshape(freqs.shape[0], nh * 64)
    return (np.ascontiguousarray(fr.astype(NPF16)),
            np.ascontiguousarray(fi.astype(NPF16)))


def _rope_perm():
    # de-interleave (re, im) pairs within each head's 128 dims:
    # new col h*128 + s*64 + i  <-  old col h*128 + 2*i + s
    perm = np.empty(GD, np.int64)
    for h in range(GH):
        for i in range(64):
            for s in range(2):
                perm[h * 128 + s * 64 + i] = h * 128 + 2 * i + s
    return perm


def make_in_maps_tp(x, cam_emb, render_emb, freqs_x, freqs_cam, freqs_render,
                    wq, bq, wk, bk, wv, bv, wkr, bkr, wvr, bvr, wo, bo, gq, gk):
    for b in (bq, bk, bv, bkr, bvr, bo):
        assert np.abs(np.asarray(b)).max() == 0.0, "nonzero bias unsupported"
    assert np.allclose(np.asarray(gq), 1.0) and np.allclose(np.asarray(gk), 1.0), \
        "non-unit rmsnorm gains unsupported"

    def wT(w):
        return np.asarray(w).T.astype(NPF16)

    wqT, wkT, wvT = wT(wq), wT(wk), wT(wv)
    wkrT, wvrT, woT = wT(wkr), wT(wvr), wT(wo)
    frq, fiq = _expand_freqs(np.asarray(freqs_x))
    frc, fic = _expand_freqs(np.asarray(freqs_cam))
    frr, fir = _expand_freqs(np.asarray(freqs_render))
    perm = _rope_perm()

    x = np.asarray(x)
    cam = np.asarray(cam_emb)
    ren = np.asarray(render_emb)
    xT = [np.ascontiguousarray(x[b].T.astype(NPF16)) for b in range(2)]
    camT = [np.ascontiguousarray(cam[b].T.astype(NPF16)) for b in range(2)]
    renT = [np.ascontiguousarray(ren[b].T.astype(NPF16)) for b in range(2)]
    in_maps = []
    for c in range(N_CORES):
        b, g = divmod(c, 4)
        gs = slice(g * GD, (g + 1) * GD)
        m = {
            "xT": xT[b], "camT": camT[b], "renT": renT[b],
            "wqTg": np.ascontiguousarray(wqT[:, gs][:, perm]),
            "wkTg": np.ascontiguousarray(wkT[:, gs][:, perm]),
            "wvTg": np.ascontiguousarray(wvT[:, gs]),
            "wkrTg": np.ascontiguousarray(wkrT[:, gs][:, perm]),
            "wvrTg": np.ascontiguousarray(wvrT[:, gs]),
            "woTg": np.ascontiguousarray(woT[gs, :]),
            "frq": frq, "fiq": fiq,
            "frc": frc, "fic": fic, "frr": frr, "fir": fir,
        }
        in_maps.append(m)
    return in_maps


def kernel(**inputs):
    nc = build_program()
    in_maps = make_in_maps_tp(**inputs)
    res = run_bass_kernel_spmd(nc, in_maps, core_ids=list(range(N_CORES)))
    x = np.asarray(inputs["x"])
    out = np.empty((x.shape[0], x.shape[1], DIM), dtype=np.float32)
    for b in range(2):
        acc = res.results[4 * b]["out"].astype(np.float32)
        for g in range(1, 4):
            acc = acc + res.results[4 * b + g]["out"].astype(np.float32)
        out[b] = acc
    out += np.asarray(inputs["bo"])[None, None, :]
    return out


def _make_timed_runner(nc, in_maps):
    """Reusable jitted SPMD callable with device-resident inputs."""
    import jax
    from jax.experimental.shard_map import shard_map
    from jax.sharding import Mesh, PartitionSpec, NamedSharding
    from concourse import bass2jax, mybir as mb

    bass2jax.install_neuronx_cc_hook()

    in_names, out_names, out_avals = [], [], []
    partition_name = nc.partition_id_tensor.name if nc.partition_id_tensor else None
    for alloc in nc.m.functions[0].allocations:
        if not isinstance(alloc, mb.MemoryLocationSet):
            continue
        name = alloc.memorylocations[0].name
        if alloc.kind == "ExternalInput":
            if name != partition_name:
                in_names.append(name)
        elif alloc.kind == "ExternalOutput":
            shape = tuple(alloc.tensor_shape)
            dtype = mb.dt.np(alloc.dtype)
            out_names.append(name)
            out_avals.append(jax.core.ShapedArray(shape, dtype))
    n_params = len(in_names)
    all_names = list(in_names) + list(out_names)
    if partition_name is not None:
        all_names.append(partition_name)

    def _body(*args):
        operands = list(args)
        if partition_name is not None:
            operands.append(bass2jax.partition_id_tensor())
        outs = bass2jax._bass_exec_p.bind(
            *operands,
            out_avals=tuple(out_avals),
            in_names=tuple(all_names),
            out_names=tuple(out_names),
            lowering_input_output_aliases=(),
            sim_require_finite=True,
            sim_require_nnan=True,
            nc=nc,
        )
        return tuple(outs)

    devices = jax.devices()[:N_CORES]
    mesh = Mesh(np.asarray(devices), ("core",))
    in_specs = (PartitionSpec("core"),) * (n_params + len(out_names))
    out_specs = (PartitionSpec("core"),) * len(out_names)
    sharded = jax.jit(
        shard_map(_body, mesh=mesh, in_specs=in_specs, out_specs=out_specs,
                  check_rep=False),
        keep_unused=True,
    )
    sharding = NamedSharding(mesh, PartitionSpec("core"))
    concat_in = [
        jax.device_put(
            np.concatenate([np.asarray(in_maps[c][nm]) for c in range(N_CORES)],
                           axis=0),
            sharding,
        )
        for nm in in_names
    ]
    for av in out_avals:
        concat_in.append(
            jax.device_put(
                np.zeros((N_CORES * av.shape[0], *av.shape[1:]), av.dtype), sharding
            )
        )
    return sharded, concat_in


def bench(inputs, iters=10):
    """Return per-execution device time in ns, amortized over `iters` runs."""
    import time
    import jax

    nc = build_program()
    in_maps = make_in_maps_tp(**inputs)
    fn, dev_in = _make_timed_runner(nc, in_maps)
    outs = fn(*dev_in)
    jax.block_until_ready(outs)
    t0 = time.perf_counter()
    for _ in range(iters):
        outs = fn(*dev_in)
    jax.block_until_ready(outs)
    dt = (time.perf_counter() - t0) / iters
    return dt * 1e9
